# revision 15
# baseline (speedup 1.0000x reference)
"""Bidirectional Conv-Mamba block for Trainium2, 8-core batch-parallel.

kernel(**inputs) takes FULL unsharded inputs, shards batch over 8 NeuronCores
(1 batch element per core), returns the FULL [8, 2048, 256] float32 output.
"""

import os
import sys

for _p in ("/opt/trn_rl_repo", "/root/.axon_site/_ro/trn_rl_repo"):
    if os.path.isdir(_p) and _p not in sys.path:
        sys.path.append(_p)

import numpy as np
import ml_dtypes

B = 8
L = 2048
HL = 1024        # time half
D = 256
DI = 512
N = 32
R = 16
NT = L // 128
DD = D // 128    # 2
DT = DI // 128   # 4
CH = L // 512    # 4
W2 = 2 * HL + 1  # scanned columns per window

_BUILT = None


def _bf(x):
    return np.ascontiguousarray(x.astype(ml_dtypes.bfloat16))


def _f32(x):
    return np.ascontiguousarray(x.astype(np.float32))


def _pack_cols(v, ntiles):
    k = v.shape[1]
    return np.ascontiguousarray(v.reshape(ntiles, 128, k).transpose(1, 0, 2))


def _host_prep(inputs):
    p = {k: np.asarray(v, dtype=np.float32) for k, v in inputs.items()}
    out = {}
    out["rms_w_row"] = _f32(p["rms_w"].reshape(1, D))

    lconv = np.zeros((128, 3 * DD, 128), np.float32)
    for k in range(3):
        for dd in range(DD):
            lconv[:, k * DD + dd, :][np.arange(128), np.arange(128)] = \
                p["local_conv_w"][dd * 128:(dd + 1) * 128, 0, k]
    out["lconv_diag"] = _bf(lconv)
    out["lconv_b"] = _f32(_pack_cols(p["local_conv_b"].reshape(D, 1), DD))
    out["lnc_w"] = _f32(_pack_cols(p["lnc_w"].reshape(D, 1), DD))
    out["lnc_b"] = _f32(_pack_cols(p["lnc_b"].reshape(D, 1), DD))

    out["inproj_lhsT"] = _bf(_pack_cols(p["in_proj_w"].T, DD))

    mconv = np.zeros((128, 4 * DT, 128), np.float32)
    for k in range(4):
        for dt in range(DT):
            mconv[:, k * DT + dt, :][np.arange(128), np.arange(128)] = \
                p["conv1d_w"][dt * 128:(dt + 1) * 128, 0, k]
    out["mconv_diag"] = _bf(mconv)
    out["mconv_b"] = _f32(_pack_cols(p["conv1d_b"].reshape(DI, 1), DT))

    xp = np.zeros((DI, 96), np.float32)
    xp[:, 0:R] = p["x_proj_w"].T[:, 0:R]
    xp[:, 32:96] = p["x_proj_w"].T[:, R:80]
    out["xproj_lhsT"] = _f32(_pack_cols(xp, DT))
    out["dtproj_lhsT"] = _f32(p["dt_proj_w"].T)
    out["dtproj_b"] = _f32(_pack_cols(p["dt_proj_b"].reshape(DI, 1), DT))
    out["A_cols"] = _f32(_pack_cols(-np.exp(p["A_log"]), DT))
    out["ssm_D"] = _f32(_pack_cols(p["ssm_D"].reshape(DI, 1), DT))
    out["outproj_lhsT"] = _bf(_pack_cols(p["out_proj_w"].T, DT))
    out["lnp_w"] = _f32(_pack_cols(p["lnp_w"].reshape(2 * D, 1), 2 * DD))
    out["lnp_b"] = _f32(_pack_cols(p["lnp_b"].reshape(2 * D, 1), 2 * DD))

    psel = np.zeros((128, DD * 2 * 3, 128), np.float32)
    for g in range(D):
        gt, gl = divmod(g, 128)
        for j in range(2):
            c = 2 * g + j
            ct, cl = divmod(c, 128)
            jj = ct - 2 * gt
            for k in range(3):
                psel[cl, (gt * 2 + jj) * 3 + k, gl] = p["post_conv_w"][g, j, k]
    out["pconv_sel"] = _bf(psel)
    out["pconv_b"] = _f32(_pack_cols(p["post_conv_b"].reshape(D, 1), DD))

    out["mlp1_lhsT"] = _bf(_pack_cols(p["mlp_w1"].T, DD))
    out["mlp1_b"] = _f32(_pack_cols(p["mlp_b1"].reshape(4 * D, 1), 8))
    w = np.exp(p["branch_logits"] - p["branch_logits"].max())
    w = w / w.sum()
    out["mlp2_lhsT"] = _bf(_pack_cols(p["mlp_w2"].T * w[1], 8))
    out["mlp2_b"] = _f32(_pack_cols(p["mlp_b2"].reshape(D, 1) * w[1], DD))
    out["w0_col"] = _f32(np.full((128, 1), w[0], np.float32))

    out["ident_bf"] = _bf(np.eye(128, dtype=np.float32))
    out["ident_f32"] = _f32(np.eye(128, dtype=np.float32))
    out["ones_f32r"] = _f32(np.ones((128, 1), np.float32))
    out["ones_bf"] = _bf(np.ones((128, 1), np.float32))
    return out


def _build():
    import contextlib

    import concourse.bacc as bacc
    import concourse.bass as bass
    import concourse.tile as tile
    from concourse import mybir

    F = mybir.dt.float32
    FR = mybir.dt.float32r
    BF = mybir.dt.bfloat16
    MUL = mybir.AluOpType.mult
    ADD = mybir.AluOpType.add
    SUB = mybir.AluOpType.subtract
    AF = mybir.ActivationFunctionType

    nc = bacc.Bacc("TRN2", target_bir_lowering=False, debug=False, num_devices=B)

    def param(name, shape, dtype=F):
        return nc.declare_dram_parameter(name, list(shape), dtype, isOutput=False)

    x_in = param("x", [L, D])
    rms_w_row = param("rms_w_row", [1, D])
    lconv_diag = param("lconv_diag", [128, 3 * DD, 128], BF)
    lconv_b = param("lconv_b", [128, DD, 1])
    lnc_w = param("lnc_w", [128, DD, 1])
    lnc_b = param("lnc_b", [128, DD, 1])
    inproj_lhsT = param("inproj_lhsT", [128, DD, 2 * DI], BF)
    mconv_diag = param("mconv_diag", [128, 4 * DT, 128], BF)
    mconv_b = param("mconv_b", [128, DT, 1])
    xproj_lhsT = param("xproj_lhsT", [128, DT, 96], FR)
    dtproj_lhsT = param("dtproj_lhsT", [R, DI], FR)
    dtproj_b = param("dtproj_b", [128, DT, 1])
    A_cols = param("A_cols", [128, DT, N])
    ssm_D = param("ssm_D", [128, DT, 1])
    outproj_lhsT = param("outproj_lhsT", [128, DT, D], BF)
    lnp_w = param("lnp_w", [128, 2 * DD, 1])
    lnp_b = param("lnp_b", [128, 2 * DD, 1])
    pconv_sel = param("pconv_sel", [128, DD * 2 * 3, 128], BF)
    pconv_b = param("pconv_b", [128, DD, 1])
    mlp1_lhsT = param("mlp1_lhsT", [128, DD, 4 * D], BF)
    mlp1_b = param("mlp1_b", [128, 8, 1])
    mlp2_lhsT = param("mlp2_lhsT", [128, 8, D], BF)
    mlp2_b = param("mlp2_b", [128, DD, 1])
    w0_col = param("w0_col", [128, 1])
    ident_bf = param("ident_bf", [128, 128], BF)
    ident_f32 = param("ident_f32", [128, 128])
    ones_f32r = param("ones_f32r", [128, 1], FR)
    ones_bf = param("ones_bf", [128, 1], BF)

    out_dram = nc.declare_dram_parameter("out", [L, D], F, isOutput=True)
    DBG = bool(int(os.environ.get("KDBG", "0")))
    if DBG:
        dbg_sz = nc.declare_dram_parameter("dbg_sz", [128, DT, L], BF, isOutput=True)
        dbg_gf = nc.declare_dram_parameter("dbg_gf", [128, DT, L], BF, isOutput=True)
        dbg_gb = nc.declare_dram_parameter("dbg_gb", [128, DT, L], BF, isOutput=True)
        dbg_xs = nc.declare_dram_parameter("dbg_xs", [128, 2 * DD, L], BF, isOutput=True)

    bc_dram = {d: nc.dram_tensor(f"bc_{d}", [64, L], BF) for d in ("f", "b")}
    db_dram = nc.dram_tensor("db_scr", [4, 128, DT * L], BF)
    xmd_dram = {d: nc.dram_tensor(f"xmd_{d}", [128, DT * L], BF) for d in ("f", "b")}
    stat_dram = nc.dram_tensor("stat_scr", [8, L], F)
    mlpT_dram = nc.dram_tensor("mlpT_scr", [L, D], BF)

    def brow(dram_t, off, ncols):
        ap = dram_t.ap()
        return bass.AP(tensor=ap.tensor, offset=off, ap=[[0, 128], [1, ncols]])

    def strided(ap2d, off, stride, count):
        # [128, count] view of a flat [128, X] tile AP at column offset/stride
        return bass.AP(tensor=ap2d.tensor, offset=ap2d.offset + off,
                       ap=[list(ap2d.ap[0]), [stride, count]])

    def bcast2(ap2d, ncols):
        # [128, 2, ncols] view of a [128, ncols] tile, broadcast on mid dim
        return bass.AP(tensor=ap2d.tensor, offset=ap2d.offset,
                       ap=[list(ap2d.ap[0]), [0, 2], [1, ncols]])

    def win3(ap2d, th_unused=None):
        # [128, 2, 1024] data view of a flat [128, 2050] window tile
        return ap2d[:, 0:2050].rearrange("p (a l) -> p a l", a=2)[:, :, 0:HL]

    with tile.TileContext(nc) as tc, contextlib.ExitStack() as ctx:
        consts = ctx.enter_context(tc.tile_pool(name="consts", bufs=1))
        wtmp = ctx.enter_context(tc.tile_pool(name="wtmp", bufs=1))
        big = ctx.enter_context(tc.tile_pool(name="big", bufs=1))
        sm = ctx.enter_context(tc.tile_pool(name="sm", bufs=4))
        pa = ctx.enter_context(tc.tile_pool(name="pa", bufs=2))
        pbz = ctx.enter_context(tc.tile_pool(name="pbz", bufs=4))
        pbc = ctx.enter_context(tc.tile_pool(name="pbc", bufs=3))
        pw8 = ctx.enter_context(tc.tile_pool(name="pw8", bufs=1))
        ps = ctx.enter_context(tc.tile_pool(name="ps", bufs=2, space="PSUM"))
        psy = ctx.enter_context(tc.tile_pool(name="psy", bufs=1, space="PSUM"))

        def load_const(name, prm, shape, dtype, pool=consts):
            t = pool.tile(shape, dtype, tag=name, name=name)
            nc.sync.dma_start(out=t[:], in_=prm.ap())
            return t

        c_ident_bf = load_const("c_ident_bf", ident_bf, [128, 128], BF)
        c_ones_fr = load_const("c_ones_fr", ones_f32r, [128, 1], FR)
        c_ones_bf = load_const("c_ones_bf", ones_bf, [128, 1], BF)
        c_xproj = load_const("c_xproj", xproj_lhsT, [128, DT, 96], FR)
        c_dtproj = load_const("c_dtproj", dtproj_lhsT, [R, DI], FR)
        c_outproj = load_const("c_outproj", outproj_lhsT, [128, DT, D], BF)
        c_A = load_const("c_A", A_cols, [128, DT, N], F)
        c_lconv_b = load_const("c_lconv_b", lconv_b, [128, DD, 1], F)
        c_lnc_w = load_const("c_lnc_w", lnc_w, [128, DD, 1], F)
        c_lnc_b = load_const("c_lnc_b", lnc_b, [128, DD, 1], F)
        c_mconv_b = load_const("c_mconv_b", mconv_b, [128, DT, 1], F)
        c_dtproj_b = load_const("c_dtproj_b", dtproj_b, [128, DT, 1], F)
        c_ssmD = load_const("c_ssmD", ssm_D, [128, DT, 1], F)
        c_lnp_w = load_const("c_lnp_w", lnp_w, [128, 2 * DD, 1], F)
        c_lnp_b = load_const("c_lnp_b", lnp_b, [128, 2 * DD, 1], F)
        c_pconv_b = load_const("c_pconv_b", pconv_b, [128, DD, 1], F)
        c_mlp1_b = load_const("c_mlp1_b", mlp1_b, [128, 8, 1], F)
        c_mlp2_b = load_const("c_mlp2_b", mlp2_b, [128, DD, 1], F)
        c_w0 = load_const("c_w0", w0_col, [128, 1], F)
        c_rmsw = consts.tile([128, D], F, tag="c_rmsw", name="c_rmsw")
        nc.sync.dma_start(out=c_rmsw[:], in_=brow(rms_w_row, 0, D))
        eps6 = consts.tile([128, 1], F, tag="eps6", name="eps6")
        nc.vector.memset(eps6[:], 1e-6)
        eps5 = consts.tile([128, 1], F, tag="eps5", name="eps5")
        nc.vector.memset(eps5[:], 1e-5)

        def wload(prm, shape, dtype, nm, tag="wbig"):
            t = wtmp.tile(shape, dtype, tag=tag, name=nm)
            nc.sync.dma_start(out=t[:], in_=prm.ap())
            return t

        xnT_pad = big.tile([128, DD, L + 2], BF, tag="xnT8", name="xnT_pad")
        sz = big.tile([128, DT, L], BF, tag="sz16", name="sz")
        hedge = big.tile([128, 2 * N, 2], F, tag="hedge", name="hedge")

        # manual ring buffers for the scan phase
        NAW = 2
        NBW = 2
        NHW = 2
        aw_ring = [big.tile([128, 2050], BF, tag=f"aw{j}", name=f"aw{j}")
                   for j in range(NAW)]
        bw_ring = [big.tile([128, 2050], BF, tag=f"bw{j}", name=f"bw{j}")
                   for j in range(NBW)]
        hw_ring = [big.tile([128, 2050], BF, tag=f"hw{j}", name=f"hw{j}")
                   for j in range(NHW)]
        for j in range(NAW):
            nc.vector.memset(aw_ring[j][:, HL:HL + 1], 0.0)
            nc.vector.memset(aw_ring[j][:, 2049:2050], 0.0)

        # ---- A1+A2: rmsnorm fused with transpose ---------------------------
        nc.vector.memset(xnT_pad[:, :, 0:1], 0.0)
        nc.vector.memset(xnT_pad[:, :, L + 1:L + 2], 0.0)
        for tt in range(NT):
            x_t = pa.tile([128, 512], F, tag="a_t", name="x_t")
            nc.gpsimd.dma_start(out=x_t[:, :D], in_=x_in.ap()[tt * 128:(tt + 1) * 128, :])
            sq = pbz.tile([128, 512], BF, tag="sq2", name="sq", bufs=1)
            ssq = sm.tile([128, 1], F, tag="ssq", name="ssq")
            nc.scalar.activation(sq[:, :D], x_t[:, :D], AF.Square, accum_out=ssq[:])
            rsq = sm.tile([128, 1], F, tag="rsq", name="rsq")
            nc.scalar.activation(rsq[:], ssq[:], AF.Sqrt, bias=eps6[:], scale=1.0 / D)
            nc.vector.reciprocal(rsq[:], rsq[:])
            xn_t = pbz.tile([128, 512], BF, tag="xn_t", name="xn_t", bufs=1)
            nc.scalar.activation(xn_t[:, :D], x_t[:, :D], AF.Copy, scale=rsq[:])
            nc.vector.tensor_tensor(out=xn_t[:, :D], in0=xn_t[:, :D], in1=c_rmsw[:], op=MUL)
            for dd in range(DD):
                pt = ps.tile([128, 512], BF, tag="convB", name="pt", bufs=1)
                nc.tensor.transpose(pt[:, :128], xn_t[:, dd * 128:(dd + 1) * 128], c_ident_bf[:])
                nc.scalar.copy(xnT_pad[:, dd, 1 + tt * 128:1 + (tt + 1) * 128], pt[:, :128])

        # ---- A3: local conv + square --------------------------------------
        c_lconv = wload(lconv_diag, [128, 3 * DD, 128], BF, "c_lconv")
        xc = big.tile([128, DD, L], BF, tag="beta16x", name="xc")
        xc2 = big.tile([128, DD, L], BF, tag="xc2mix", name="xc2")
        for dd in range(DD):
            for c in range(CH):
                pc = ps.tile([128, 512], F, tag="convA", name="pc")
                for k in range(3):
                    nc.tensor.matmul(pc[:], c_lconv[:, k * DD + dd, :],
                                     xnT_pad[:, dd, k + c * 512:k + c * 512 + 512],
                                     start=(k == 0), stop=(k == 2))
                nc.scalar.activation(xc[:, dd, c * 512:(c + 1) * 512], pc[:], AF.Identity,
                                     bias=c_lconv_b[:, dd, :])
                nc.scalar.activation(xc2[:, dd, c * 512:(c + 1) * 512], pc[:], AF.Square,
                                     bias=c_lconv_b[:, dd, :])

        # ---- A4: LNc stats + apply ----------------------------------------
        for si, src in ((0, xc), (1, xc2)):
            for c in range(CH):
                pst = ps.tile([1, 512], F, tag="stat", name="pst", bufs=1)
                for dd in range(DD):
                    nc.tensor.matmul(pst[:], c_ones_bf[:],
                                     src[:, dd, c * 512:(c + 1) * 512],
                                     start=(dd == 0), stop=(dd == DD - 1))
                sev = sm.tile([1, 512], F, tag="sev", name="sev")
                nc.scalar.copy(sev[:], pst[:])
                nc.sync.dma_start(out=stat_dram.ap()[si, c * 512:(c + 1) * 512].unsqueeze(0),
                                  in_=sev[:])
        st = sm.tile([128, 2, 16], F, tag="st", name="st")
        nc.sync.dma_start(out=st[:], in_=stat_dram.ap()[0:2, :].rearrange("s (p i) -> p s i", p=128))
        mu = sm.tile([128, 16], F, tag="mu", name="mu")
        nc.vector.tensor_scalar_mul(mu[:], st[:, 0, :], 1.0 / D)
        var = sm.tile([128, 16], F, tag="var", name="var")
        nc.vector.tensor_tensor(out=var[:], in0=mu[:], in1=mu[:], op=MUL)
        qv = sm.tile([128, 16], F, tag="qv", name="qv")
        nc.vector.tensor_scalar_mul(qv[:], st[:, 1, :], 1.0 / D)
        nc.vector.tensor_tensor(out=var[:], in0=qv[:], in1=var[:], op=SUB)
        rstd = sm.tile([128, 16], F, tag="rstd", name="rstd")
        nc.scalar.activation(rstd[:], var[:], AF.Sqrt, bias=eps5[:])
        nc.vector.reciprocal(rstd[:], rstd[:])
        nu = sm.tile([128, 16], F, tag="nu", name="nu")
        nc.vector.tensor_tensor(out=nu[:], in0=mu[:], in1=rstd[:], op=MUL)
        nc.sync.dma_start(out=stat_dram.ap()[2, :].rearrange("(p i) -> p i", p=128), in_=rstd[:])
        nc.sync.dma_start(out=stat_dram.ap()[3, :].rearrange("(p i) -> p i", p=128), in_=nu[:])
        rs_bc = pw8.tile([128, L], BF, tag="t8w", name="rs_bc")
        nc.gpsimd.dma_start(out=rs_bc[:], in_=brow(stat_dram, 2 * L, L))
        nu_bc = pw8.tile([128, L], BF, tag="t8x", name="nu_bc")
        nc.gpsimd.dma_start(out=nu_bc[:], in_=brow(stat_dram, 3 * L, L))

        xcsT = big.tile([128, DD, L], BF, tag="delta16x", name="xcsT")
        for dd in range(DD):
            for hh in range(2):
                t1 = pw8.tile([128, HL], BF, tag="t8y", name="t1")
                hsl = slice(hh * HL, (hh + 1) * HL)
                nc.vector.tensor_tensor(out=t1[:], in0=xc[:, dd, hsl], in1=rs_bc[:, hsl], op=MUL)
                nc.vector.tensor_tensor(out=t1[:], in0=t1[:], in1=nu_bc[:, hsl], op=SUB)
                nc.scalar.activation(xcsT[:, dd, hsl], t1[:], AF.Silu, bias=c_lnc_b[:, dd, :],
                                     scale=c_lnc_w[:, dd, :])

        # ---- A5: in_proj ----------------------------------------------------
        c_inproj = wload(inproj_lhsT, [128, DD, 2 * DI], BF, "c_inproj")
        xmT_pad = big.tile([128, DT, L + 6], BF, tag="g16", name="xmT_pad")
        nc.vector.memset(xmT_pad[:, :, 0:3], 0.0)
        nc.vector.memset(xmT_pad[:, :, L + 3:L + 6], 0.0)
        for m in range(8):
            for c in range(CH):
                pc = ps.tile([128, 512], F, tag="convA", name="pc2")
                for kk in range(DD):
                    nc.tensor.matmul(pc[:], c_inproj[:, kk, m * 128:(m + 1) * 128],
                                     xcsT[:, kk, c * 512:(c + 1) * 512],
                                     start=(kk == 0), stop=(kk == DD - 1))
                if m < 4:
                    nc.scalar.copy(xmT_pad[:, m, 3 + c * 512:3 + (c + 1) * 512], pc[:])
                else:
                    nc.scalar.activation(sz[:, m - 4, c * 512:(c + 1) * 512], pc[:], AF.Silu)

        # ---- A6(d): mamba conv + x_proj + dt/beta/xmd ----------------------
        c_mconv = wload(mconv_diag, [128, 4 * DT, 128], BF, "c_mconv2", tag="wbig2")

        def stage_a6(d):
            xm_bf = big.tile([128, DT, L], BF, tag="xs16", name=f"xm_bf_{d}")
            dtT = big.tile([R, L], FR, tag="dtT", name=f"dtT_{d}")
            for c in range(CH):
                xm_fr = pa.tile([128, DT, 512], FR, tag="xm_fr", name="xm_fr", bufs=1)
                for dt in range(DT):
                    pc = ps.tile([128, 512], F, tag="convA", name="pc3")
                    for k in range(4):
                        off = (k if d == "f" else 6 - k) + c * 512
                        nc.tensor.matmul(pc[:], c_mconv[:, k * DT + dt, :],
                                         xmT_pad[:, dt, off:off + 512],
                                         start=(k == 0), stop=(k == 3))
                    nc.scalar.activation(xm_fr[:, dt, :], pc[:], AF.Silu, bias=c_mconv_b[:, dt, :])
                    nc.scalar.activation(xm_bf[:, dt, c * 512:(c + 1) * 512], pc[:], AF.Silu,
                                         bias=c_mconv_b[:, dt, :])
                psx = ps.tile([96, 512], F, tag="stat", name="psx", bufs=1)
                for dt in range(DT):
                    nc.tensor.matmul(psx[:], c_xproj[:, dt, :], xm_fr[:, dt, :],
                                     start=(dt == 0), stop=(dt == DT - 1))
                nc.scalar.copy(dtT[:, c * 512:(c + 1) * 512], psx[0:R, :])
                bc_ev = pbz.tile([64, 512], BF, tag="bc_ev", name="bc_ev", bufs=1)
                nc.scalar.copy(bc_ev[0:32, :], psx[32:64, :])
                nc.scalar.copy(bc_ev[32:64, :], psx[64:96, :])
                nc.sync.dma_start(out=bc_dram[d].ap()[:, c * 512:(c + 1) * 512], in_=bc_ev[:])
            drow = 0 if d == "f" else 2
            # batch Exp then Ln (softplus) in two half-rounds - avoids ACT
            # table thrash while keeping the staging tile at 8KB/partition
            for mh in range(2):
                e1t = big.tile([128, 2, L], BF, tag="xc2mix", name=f"e1t_{d}{mh}")
                for mi in range(2):
                    m = mh * 2 + mi
                    for c in range(CH):
                        pc = ps.tile([128, 512], F, tag="convA", name="pc4")
                        nc.tensor.matmul(pc[:], c_dtproj[:, m * 128:(m + 1) * 128],
                                         dtT[:, c * 512:(c + 1) * 512], start=True, stop=True)
                        nc.scalar.activation(e1t[:, mi, c * 512:(c + 1) * 512], pc[:], AF.Exp,
                                             bias=c_dtproj_b[:, m, :])
                for mi in range(2):
                    m = mh * 2 + mi
                    for c in range(CH):
                        dev = pbz.tile([128, 512], BF, tag="dev", name="dev", bufs=2)
                        nc.scalar.activation(dev[:], e1t[:, mi, c * 512:(c + 1) * 512], AF.Ln,
                                             bias=1.0)
                        nc.sync.dma_start(
                            out=db_dram.ap()[drow, :, m * L + c * 512:m * L + (c + 1) * 512],
                            in_=dev[:])
                        bev = pbz.tile([128, 512], BF, tag="bev", name="bev", bufs=2)
                        nc.vector.tensor_tensor(out=bev[:], in0=dev[:],
                                                in1=xm_bf[:, m, c * 512:(c + 1) * 512], op=MUL)
                        nc.sync.dma_start(
                            out=db_dram.ap()[drow + 1, :, m * L + c * 512:m * L + (c + 1) * 512],
                            in_=bev[:])
            for dt in range(DT):
                for c in range(CH):
                    xev = pbz.tile([128, 512], BF, tag="xev", name="xev", bufs=2)
                    nc.scalar.activation(xev[:], xm_bf[:, dt, c * 512:(c + 1) * 512],
                                         AF.Copy, scale=c_ssmD[:, dt, :])
                    nc.sync.dma_start(
                        out=xmd_dram[d].ap()[:, dt * L + c * 512:dt * L + (c + 1) * 512],
                        in_=xev[:])

        # ---- scan phase: one (direction, pair) window group ----------------
        gtiles = {}
        ring_idx = [0, 0, 0]  # aw, bw, hw

        def stage_scan_pair(d, pair):
            drow = 0 if d == "f" else 2
            if pair == 0:
                tag = "gdir" if d == "f" else "g16"
                gtiles[d] = big.tile([128, DT, L], BF, tag=tag, name=f"g_{d}")
            g = gtiles[d]
            delta_p = big.tile([128, 2, L], BF, tag="delta16x", name=f"delta_{d}{pair}")
            beta_p = big.tile([128, 2, L], BF, tag="beta16x", name=f"beta_{d}{pair}")
            ths = (0, 1) if d == "f" else (1, 0)
            for sh in (ths[0], ths[1]):
                hs = slice(sh * HL, (sh + 1) * HL)
                nc.sync.dma_start(out=delta_p[:, :, hs], in_=db_dram.ap()[
                    drow, :, 2 * pair * L:(2 * pair + 2) * L].rearrange(
                        "p (a l) -> p a l", a=2)[:, :, hs])
                nc.sync.dma_start(out=beta_p[:, :, hs], in_=db_dram.ap()[
                    drow + 1, :, 2 * pair * L:(2 * pair + 2) * L].rearrange(
                        "p (a l) -> p a l", a=2)[:, :, hs])
            for thi, th in enumerate(ths):
                # prezero bw link columns at window start (first th only needs
                # 0-link; second th overwrites the link with the carried state)
                if thi == 0:
                    for j in range(NBW):
                        nc.vector.memset(bw_ring[j][:, HL:HL + 1], 0.0)
                        nc.vector.memset(bw_ring[j][:, 2049:2050], 0.0)
                yp = psy.tile([128, 2, HL], F, tag="ypair", name="yp")
                xmd_t = {}
                for i in range(2):
                    dt = pair * 2 + i
                    xmd_t[i] = pbz.tile([128, HL], BF, tag=f"xmd{i}", name="xmd_c", bufs=1)
                    nc.gpsimd.dma_start(
                        out=xmd_t[i][:],
                        in_=xmd_dram[d].ap()[:, dt * L + th * HL:dt * L + (th + 1) * HL])
                for n in range(N):
                    slot = pair * N + n
                    B_bc = pbc.tile([128, HL], BF, tag="B_bc", name="B_bc", bufs=2)
                    nc.gpsimd.dma_start(out=B_bc[:], in_=brow(bc_dram[d], n * L + th * HL, HL))
                    C_bc = pbc.tile([128, HL], BF, tag="C_bc", name="C_bc", bufs=2)
                    nc.gpsimd.dma_start(out=C_bc[:],
                                        in_=brow(bc_dram[d], (N + n) * L + th * HL, HL))
                    aw = aw_ring[ring_idx[0] % NAW]; ring_idx[0] += 1
                    bw = bw_ring[ring_idx[1] % NBW]; ring_idx[1] += 1
                    hw = hw_ring[ring_idx[2] % NHW]; ring_idx[2] += 1
                    r_aw = win3(aw[:])
                    for i in range(2):
                        nc.scalar.activation(r_aw[:, i, :], delta_p[:, i, th * HL:(th + 1) * HL],
                                             AF.Exp, scale=c_A[:, pair * 2 + i, n:n + 1])
                    nc.vector.tensor_tensor(out=win3(bw[:]),
                                            in0=beta_p[:, :, th * HL:(th + 1) * HL],
                                            in1=bcast2(B_bc[:], HL), op=MUL)
                    if thi == 1:
                        src = hedge[:, slot, 1:2] if d == "f" else hedge[:, slot, 0:1]
                        nc.vector.tensor_copy(out=bw[:, HL:HL + 1], in_=src)
                        init = hedge[:, slot, 0:1] if d == "f" else hedge[:, slot, 1:2]
                    else:
                        init = 0.0
                    if d == "f":
                        nc.vector.tensor_tensor_scan(hw[:, 0:W2], aw[:, 0:W2], bw[:, 0:W2],
                                                     init, MUL, ADD)
                    else:
                        nc.vector.tensor_tensor_scan(hw[:, 0:W2][:, ::-1], aw[:, 0:W2][:, ::-1],
                                                     bw[:, 0:W2][:, ::-1], init, MUL, ADD)
                    if thi == 0:
                        off = 1023 if d == "f" else 0
                        nc.vector.tensor_copy(out=hedge[:, slot, 0:2],
                                              in_=strided(hw[:], off, 1025, 2))
                    nc.vector.tensor_tensor(out=win3(hw[:]), in0=win3(hw[:]),
                                            in1=bcast2(C_bc[:], HL), op=MUL)
                    for i in range(2):
                        for c2 in range(2):
                            base = i * 1025 + c2 * 512
                            nc.tensor.matmul(yp[:, i, c2 * 512:(c2 + 1) * 512], c_ident_bf[:],
                                             hw[:, base:base + 512],
                                             start=(n == 0), stop=False)
                for i in range(2):
                    for c2 in range(2):
                        nc.tensor.matmul(yp[:, i, c2 * 512:(c2 + 1) * 512], c_ident_bf[:],
                                         xmd_t[i][:, c2 * 512:(c2 + 1) * 512],
                                         start=False, stop=True)
                nc.vector.tensor_tensor(
                    out=g[:, 2 * pair:2 * pair + 2, th * HL:(th + 1) * HL],
                    in0=yp[:, :, :],
                    in1=sz[:, 2 * pair:2 * pair + 2, th * HL:(th + 1) * HL], op=MUL)

        # ---- C/E: out_proj + residual --------------------------------------
        def stage_outproj(d, xs):
            g = gtiles[d]
            base = 0 if d == "f" else DD
            for m in range(DD):
                for c in range(CH):
                    pc = ps.tile([128, 512], F, tag="convA", name="pc5")
                    for dt in range(DT):
                        nc.tensor.matmul(pc[:], c_outproj[:, dt, m * 128:(m + 1) * 128],
                                         g[:, dt, c * 512:(c + 1) * 512],
                                         start=(dt == 0), stop=False)
                    nc.tensor.matmul(pc[:], c_ident_bf[:],
                                     xnT_pad[:, m, 1 + c * 512:1 + (c + 1) * 512],
                                     start=False, stop=True)
                    nc.scalar.copy(xs[:, base + m, c * 512:(c + 1) * 512], pc[:])

        # ---- F3: MLP -> mlpT rows in DRAM ----------------------------------
        def stage_mlp():
            c_mlp1 = wload(mlp1_lhsT, [128, DD, 4 * D], BF, "c_mlp1")
            c_mlp2 = wload(mlp2_lhsT, [128, 8, D], BF, "c_mlp2", tag="wbig2")
            for c in range(CH):
                h1 = big.tile([128, 8, 512], BF, tag="xc2mix", name="h1")
                for m in range(8):
                    pc = ps.tile([128, 512], F, tag="convA", name="pc7")
                    for kk in range(DD):
                        nc.tensor.matmul(pc[:], c_mlp1[:, kk, m * 128:(m + 1) * 128],
                                         xnT_pad[:, kk, 1 + c * 512:1 + (c + 1) * 512],
                                         start=(kk == 0), stop=(kk == DD - 1))
                    nc.scalar.activation(h1[:, m, :], pc[:], AF.Gelu, bias=c_mlp1_b[:, m, :])
                mlpc = pbz.tile([128, DD, 512], BF, tag="b_t", name="mlpc", bufs=2)
                for m2 in range(DD):
                    pc = ps.tile([128, 512], F, tag="convA", name="pc8")
                    for mk in range(8):
                        nc.tensor.matmul(pc[:], c_mlp2[:, mk, m2 * 128:(m2 + 1) * 128],
                                         h1[:, mk, :], start=(mk == 0), stop=(mk == 7))
                    nc.scalar.activation(mlpc[:, m2, :], pc[:], AF.Identity,
                                         bias=c_mlp2_b[:, m2, :])
                for q in range(4):
                    tt = c * 4 + q
                    mt = pbz.tile([128, 512], BF, tag="z_t", name="mt", bufs=2)
                    for m2 in range(DD):
                        pt3 = ps.tile([128, 512], BF, tag="convB", name="pt3", bufs=1)
                        nc.tensor.transpose(pt3[:, :128], mlpc[:, m2, q * 128:(q + 1) * 128],
                                            c_ident_bf[:])
                        nc.scalar.copy(mt[:, m2 * 128:(m2 + 1) * 128], pt3[:, :128])
                    nc.sync.dma_start(out=mlpT_dram.ap()[tt * 128:(tt + 1) * 128, :],
                                      in_=mt[:, :D])

        # ================= emission schedule =================
        stage_a6("f")
        stage_scan_pair("f", 0)
        stage_a6("b")
        xs = big.tile([128, 2 * DD, L], BF, tag="xs16", name="xs")
        stage_scan_pair("f", 1)
        stage_outproj("f", xs)
        stage_scan_pair("b", 0)
        stage_mlp()
        stage_scan_pair("b", 1)
        stage_outproj("b", xs)
        if DBG:
            nc.sync.dma_start(out=dbg_sz.ap(), in_=sz[:])
            nc.sync.dma_start(out=dbg_gf.ap(), in_=gtiles["f"][:])
            nc.sync.dma_start(out=dbg_gb.ap(), in_=gtiles["b"][:])
            nc.sync.dma_start(out=dbg_xs.ap(), in_=xs[:])

        # ---- F1: LNp ---------------------------------------------------------
        xs2 = big.tile([128, 2 * DD, L], BF, tag="gdir", name="xs2")
        for dt in range(2 * DD):
            nc.scalar.activation(xs2[:, dt, :], xs[:, dt, :], AF.Square)
        for si, src in ((4, xs), (5, xs2)):
            for c in range(CH):
                pst = ps.tile([1, 512], F, tag="stat", name="pst2", bufs=1)
                for dt in range(2 * DD):
                    nc.tensor.matmul(pst[:], c_ones_bf[:], src[:, dt, c * 512:(c + 1) * 512],
                                     start=(dt == 0), stop=(dt == 2 * DD - 1))
                sev2 = sm.tile([1, 512], F, tag="sev", name="sev2")
                nc.scalar.copy(sev2[:], pst[:])
                nc.sync.dma_start(out=stat_dram.ap()[si, c * 512:(c + 1) * 512].unsqueeze(0),
                                  in_=sev2[:])
        stp = sm.tile([128, 2, 16], F, tag="st", name="stp")
        nc.sync.dma_start(out=stp[:], in_=stat_dram.ap()[4:6, :].rearrange("s (p i) -> p s i", p=128))
        mup = sm.tile([128, 16], F, tag="mu", name="mup")
        nc.vector.tensor_scalar_mul(mup[:], stp[:, 0, :], 1.0 / (2 * D))
        varp = sm.tile([128, 16], F, tag="var", name="varp")
        nc.vector.tensor_tensor(out=varp[:], in0=mup[:], in1=mup[:], op=MUL)
        qp = sm.tile([128, 16], F, tag="qv", name="qp")
        nc.vector.tensor_scalar_mul(qp[:], stp[:, 1, :], 1.0 / (2 * D))
        nc.vector.tensor_tensor(out=varp[:], in0=qp[:], in1=varp[:], op=SUB)
        rstdp = sm.tile([128, 16], F, tag="rstd", name="rstdp")
        nc.scalar.activation(rstdp[:], varp[:], AF.Sqrt, bias=eps5[:])
        nc.vector.reciprocal(rstdp[:], rstdp[:])
        nup = sm.tile([128, 16], F, tag="nu", name="nup")
        nc.vector.tensor_tensor(out=nup[:], in0=mup[:], in1=rstdp[:], op=MUL)
        nc.sync.dma_start(out=stat_dram.ap()[6, :].rearrange("(p i) -> p i", p=128), in_=rstdp[:])
        nc.sync.dma_start(out=stat_dram.ap()[7, :].rearrange("(p i) -> p i", p=128), in_=nup[:])
        rsp_bc = pw8.tile([128, L], BF, tag="t8w", name="rsp_bc")
        nc.gpsimd.dma_start(out=rsp_bc[:], in_=brow(stat_dram, 6 * L, L))
        nup_bc = pw8.tile([128, L], BF, tag="t8x", name="nup_bc")
        nc.gpsimd.dma_start(out=nup_bc[:], in_=brow(stat_dram, 7 * L, L))

        xsn_pad = big.tile([128, 2 * DD, L + 2], BF, tag="g16", name="xsn_pad")
        nc.vector.memset(xsn_pad[:, :, 0:1], 0.0)
        nc.vector.memset(xsn_pad[:, :, L + 1:L + 2], 0.0)
        for dt in range(2 * DD):
            t1 = pw8.tile([128, L], BF, tag="t8y", name="t1p")
            nc.vector.tensor_tensor(out=t1[:], in0=xs[:, dt, :], in1=rsp_bc[:], op=MUL)
            nc.vector.tensor_tensor(out=t1[:], in0=t1[:], in1=nup_bc[:], op=SUB)
            nc.scalar.activation(xsn_pad[:, dt, 1:1 + L], t1[:],
                                 AF.Identity, bias=c_lnp_b[:, dt, :], scale=c_lnp_w[:, dt, :])

        # ---- F2: post conv ---------------------------------------------------
        c_psel = wload(pconv_sel, [128, DD * 2 * 3, 128], BF, "c_psel")
        mixer = big.tile([128, DD, L], BF, tag="xc2mix", name="mixer")
        for gt in range(DD):
            for c in range(CH):
                pc = ps.tile([128, 512], F, tag="convA", name="pc6")
                first = True
                for jj in range(2):
                    ct = 2 * gt + jj
                    for k in range(3):
                        nc.tensor.matmul(pc[:], c_psel[:, (gt * 2 + jj) * 3 + k, :],
                                         xsn_pad[:, ct, k + c * 512:k + c * 512 + 512],
                                         start=first, stop=(jj == 1 and k == 2))
                        first = False
                nc.scalar.activation(mixer[:, gt, c * 512:(c + 1) * 512], pc[:], AF.Silu,
                                     bias=c_pconv_b[:, gt, :])

        # ---- F4: transpose back + combine + store ----------------------------
        for tt in range(NT):
            mixT = pbz.tile([128, HL], BF, tag="b_t", name="mixT", bufs=2)
            mlpT = pbz.tile([128, HL], BF, tag="b_t", name="mlpT", bufs=2)
            nc.sync.dma_start(out=mlpT[:, :D], in_=mlpT_dram.ap()[tt * 128:(tt + 1) * 128, :])
            for dd in range(DD):
                pt = ps.tile([128, 512], BF, tag="convB", name="ptb", bufs=1)
                nc.tensor.transpose(pt[:, :128], mixer[:, dd, tt * 128:(tt + 1) * 128],
                                    c_ident_bf[:])
                nc.scalar.copy(mixT[:, dd * 128:(dd + 1) * 128], pt[:, :128])
            x_t2 = pa.tile([128, 512], F, tag="a_t", name="x_t2")
            nc.sync.dma_start(out=x_t2[:, :D], in_=x_in.ap()[tt * 128:(tt + 1) * 128, :])
            o_t = pw8.tile([128, 512], F, tag="t8y", name="o_t")
            nc.vector.tensor_scalar_mul(o_t[:, :D], mixT[:, :D], c_w0[:])
            nc.vector.tensor_tensor(out=o_t[:, :D], in0=o_t[:, :D], in1=x_t2[:, :D], op=ADD)
            nc.vector.tensor_tensor(out=o_t[:, :D], in0=o_t[:, :D], in1=mlpT[:, :D], op=ADD)
            nc.sync.dma_start(out=out_dram.ap()[tt * 128:(tt + 1) * 128, :], in_=o_t[:, :D])

    nc.compile()
    return nc


def kernel(**inputs):
    global _BUILT
    from concourse.bass_utils import run_bass_kernel_spmd

    if _BUILT is None:
        _BUILT = _build()
    nc = _BUILT

    w = _host_prep(inputs)
    x = np.asarray(inputs["x"], dtype=np.float32)
    in_maps = []
    for b in range(B):
        m = dict(w)
        m["x"] = np.ascontiguousarray(x[b])
        in_maps.append(m)
    res = run_bass_kernel_spmd(nc, in_maps, list(range(B)))
    out = np.stack([res.results[b]["out"] for b in range(B)], axis=0)
    return out.astype(np.float32)


if __name__ == "__main__":
    rng = np.random.default_rng(0)
    fake = {
        "x": rng.standard_normal((B, L, D), dtype=np.float32),
        "rms_w": np.ones(D, np.float32),
        "local_conv_w": rng.standard_normal((D, 1, 3), dtype=np.float32) * 0.3,
        "local_conv_b": np.zeros(D, np.float32),
        "lnc_w": np.ones(D, np.float32),
        "lnc_b": np.zeros(D, np.float32),
        "in_proj_w": rng.standard_normal((2 * DI, D), dtype=np.float32) * 0.02,
        "conv1d_w": rng.standard_normal((DI, 1, 4), dtype=np.float32) * 0.3,
        "conv1d_b": np.zeros(DI, np.float32),
        "x_proj_w": rng.standard_normal((R + 2 * N, DI), dtype=np.float32) * 0.02,
        "dt_proj_w": rng.standard_normal((DI, R), dtype=np.float32) * 0.1,
        "dt_proj_b": np.full(DI, -4.0, np.float32),
        "A_log": np.log(np.tile(np.arange(1, N + 1, dtype=np.float32), (DI, 1))),
        "ssm_D": np.ones(DI, np.float32),
        "out_proj_w": rng.standard_normal((D, DI), dtype=np.float32) * 0.02,
        "lnp_w": np.ones(2 * D, np.float32),
        "lnp_b": np.zeros(2 * D, np.float32),
        "post_conv_w": rng.standard_normal((D, 2, 3), dtype=np.float32) * 0.3,
        "post_conv_b": np.zeros(D, np.float32),
        "mlp_w1": rng.standard_normal((4 * D, D), dtype=np.float32) * 0.02,
        "mlp_b1": np.zeros(4 * D, np.float32),
        "mlp_w2": rng.standard_normal((D, 4 * D), dtype=np.float32) * 0.02,
        "mlp_b2": np.zeros(D, np.float32),
        "branch_logits": np.array([1.0, 0.1], np.float32),
    }
    out = kernel(**fake)
    print("kernel ran, out shape", out.shape, "finite:", bool(np.isfinite(out).all()))


# revision 17
# speedup vs baseline: 1.0220x; 1.0220x over previous
"""Bidirectional Conv-Mamba block for Trainium2, 8-core batch-parallel.

kernel(**inputs) takes FULL unsharded inputs, shards batch over 8 NeuronCores
(1 batch element per core), returns the FULL [8, 2048, 256] float32 output.
"""

import os
import sys

for _p in ("/opt/trn_rl_repo", "/root/.axon_site/_ro/trn_rl_repo"):
    if os.path.isdir(_p) and _p not in sys.path:
        sys.path.append(_p)

import numpy as np
import ml_dtypes

B = 8
L = 2048
HL = 1024        # time half
D = 256
DI = 512
N = 32
R = 16
NT = L // 128
DD = D // 128    # 2
DT = DI // 128   # 4
CH = L // 512    # 4
W2 = 2 * HL + 1  # scanned columns per window

_BUILT = None


def _bf(x):
    return np.ascontiguousarray(x.astype(ml_dtypes.bfloat16))


def _f32(x):
    return np.ascontiguousarray(x.astype(np.float32))


def _pack_cols(v, ntiles):
    k = v.shape[1]
    return np.ascontiguousarray(v.reshape(ntiles, 128, k).transpose(1, 0, 2))


def _host_prep(inputs):
    p = {k: np.asarray(v, dtype=np.float32) for k, v in inputs.items()}
    out = {}
    out["rms_w_row"] = _f32(p["rms_w"].reshape(1, D))

    lconv = np.zeros((128, 3 * DD, 128), np.float32)
    for k in range(3):
        for dd in range(DD):
            lconv[:, k * DD + dd, :][np.arange(128), np.arange(128)] = \
                p["local_conv_w"][dd * 128:(dd + 1) * 128, 0, k]
    out["lconv_diag"] = _bf(lconv)
    out["lconv_b"] = _f32(_pack_cols(p["local_conv_b"].reshape(D, 1), DD))
    out["lnc_w"] = _f32(_pack_cols(p["lnc_w"].reshape(D, 1), DD))
    out["lnc_b"] = _f32(_pack_cols(p["lnc_b"].reshape(D, 1), DD))

    out["inproj_lhsT"] = _bf(_pack_cols(p["in_proj_w"].T, DD))

    mconv = np.zeros((128, 4 * DT, 128), np.float32)
    for k in range(4):
        for dt in range(DT):
            mconv[:, k * DT + dt, :][np.arange(128), np.arange(128)] = \
                p["conv1d_w"][dt * 128:(dt + 1) * 128, 0, k]
    out["mconv_diag"] = _bf(mconv)
    out["mconv_b"] = _f32(_pack_cols(p["conv1d_b"].reshape(DI, 1), DT))

    xp = np.zeros((DI, 96), np.float32)
    xp[:, 0:R] = p["x_proj_w"].T[:, 0:R]
    xp[:, 32:96] = p["x_proj_w"].T[:, R:80]
    out["xproj_lhsT"] = _f32(_pack_cols(xp, DT))
    out["dtproj_lhsT"] = _f32(p["dt_proj_w"].T)
    out["dtproj_b"] = _f32(_pack_cols(p["dt_proj_b"].reshape(DI, 1), DT))
    out["A_cols"] = _f32(_pack_cols(-np.exp(p["A_log"]), DT))
    out["ssm_D"] = _f32(_pack_cols(p["ssm_D"].reshape(DI, 1), DT))
    out["outproj_lhsT"] = _bf(_pack_cols(p["out_proj_w"].T, DT))
    out["lnp_w"] = _f32(_pack_cols(p["lnp_w"].reshape(2 * D, 1), 2 * DD))
    out["lnp_b"] = _f32(_pack_cols(p["lnp_b"].reshape(2 * D, 1), 2 * DD))

    psel = np.zeros((128, DD * 2 * 3, 128), np.float32)
    for g in range(D):
        gt, gl = divmod(g, 128)
        for j in range(2):
            c = 2 * g + j
            ct, cl = divmod(c, 128)
            jj = ct - 2 * gt
            for k in range(3):
                psel[cl, (gt * 2 + jj) * 3 + k, gl] = p["post_conv_w"][g, j, k]
    out["pconv_sel"] = _bf(psel)
    out["pconv_b"] = _f32(_pack_cols(p["post_conv_b"].reshape(D, 1), DD))

    out["mlp1_lhsT"] = _bf(_pack_cols(p["mlp_w1"].T, DD))
    out["mlp1_b"] = _f32(_pack_cols(p["mlp_b1"].reshape(4 * D, 1), 8))
    w = np.exp(p["branch_logits"] - p["branch_logits"].max())
    w = w / w.sum()
    out["mlp2_lhsT"] = _bf(_pack_cols(p["mlp_w2"].T * w[1], 8))
    out["mlp2_b"] = _f32(_pack_cols(p["mlp_b2"].reshape(D, 1) * w[1], DD))
    out["w0_col"] = _f32(np.full((128, 1), w[0], np.float32))

    out["ident_bf"] = _bf(np.eye(128, dtype=np.float32))
    out["ident_f32"] = _f32(np.eye(128, dtype=np.float32))
    out["ones_f32r"] = _f32(np.ones((128, 1), np.float32))
    out["ones_bf"] = _bf(np.ones((128, 1), np.float32))
    return out


def _build():
    import contextlib

    import concourse.bacc as bacc
    import concourse.bass as bass
    import concourse.tile as tile
    from concourse import mybir

    F = mybir.dt.float32
    FR = mybir.dt.float32r
    BF = mybir.dt.bfloat16
    MUL = mybir.AluOpType.mult
    ADD = mybir.AluOpType.add
    SUB = mybir.AluOpType.subtract
    AF = mybir.ActivationFunctionType

    nc = bacc.Bacc("TRN2", target_bir_lowering=False, debug=False, num_devices=B)

    def param(name, shape, dtype=F):
        return nc.declare_dram_parameter(name, list(shape), dtype, isOutput=False)

    x_in = param("x", [L, D])
    rms_w_row = param("rms_w_row", [1, D])
    lconv_diag = param("lconv_diag", [128, 3 * DD, 128], BF)
    lconv_b = param("lconv_b", [128, DD, 1])
    lnc_w = param("lnc_w", [128, DD, 1])
    lnc_b = param("lnc_b", [128, DD, 1])
    inproj_lhsT = param("inproj_lhsT", [128, DD, 2 * DI], BF)
    mconv_diag = param("mconv_diag", [128, 4 * DT, 128], BF)
    mconv_b = param("mconv_b", [128, DT, 1])
    xproj_lhsT = param("xproj_lhsT", [128, DT, 96], FR)
    dtproj_lhsT = param("dtproj_lhsT", [R, DI], FR)
    dtproj_b = param("dtproj_b", [128, DT, 1])
    A_cols = param("A_cols", [128, DT, N])
    ssm_D = param("ssm_D", [128, DT, 1])
    outproj_lhsT = param("outproj_lhsT", [128, DT, D], BF)
    lnp_w = param("lnp_w", [128, 2 * DD, 1])
    lnp_b = param("lnp_b", [128, 2 * DD, 1])
    pconv_sel = param("pconv_sel", [128, DD * 2 * 3, 128], BF)
    pconv_b = param("pconv_b", [128, DD, 1])
    mlp1_lhsT = param("mlp1_lhsT", [128, DD, 4 * D], BF)
    mlp1_b = param("mlp1_b", [128, 8, 1])
    mlp2_lhsT = param("mlp2_lhsT", [128, 8, D], BF)
    mlp2_b = param("mlp2_b", [128, DD, 1])
    w0_col = param("w0_col", [128, 1])
    ident_bf = param("ident_bf", [128, 128], BF)
    ident_f32 = param("ident_f32", [128, 128])
    ones_f32r = param("ones_f32r", [128, 1], FR)
    ones_bf = param("ones_bf", [128, 1], BF)

    out_dram = nc.declare_dram_parameter("out", [L, D], F, isOutput=True)
    DBG = bool(int(os.environ.get("KDBG", "0")))
    if DBG:
        dbg_sz = nc.declare_dram_parameter("dbg_sz", [128, DT, L], BF, isOutput=True)
        dbg_gf = nc.declare_dram_parameter("dbg_gf", [128, DT, L], BF, isOutput=True)
        dbg_gb = nc.declare_dram_parameter("dbg_gb", [128, DT, L], BF, isOutput=True)
        dbg_xs = nc.declare_dram_parameter("dbg_xs", [128, 2 * DD, L], BF, isOutput=True)

    bc_dram = {d: nc.dram_tensor(f"bc_{d}", [64, L], BF) for d in ("f", "b")}
    db_dram = nc.dram_tensor("db_scr", [4, 128, DT * L], BF)
    xmd_dram = {d: nc.dram_tensor(f"xmd_{d}", [128, DT * L], BF) for d in ("f", "b")}
    stat_dram = nc.dram_tensor("stat_scr", [8, L], F)
    mlpT_dram = nc.dram_tensor("mlpT_scr", [L, D], BF)

    def brow(dram_t, off, ncols):
        ap = dram_t.ap()
        return bass.AP(tensor=ap.tensor, offset=off, ap=[[0, 128], [1, ncols]])

    def strided(ap2d, off, stride, count):
        # [128, count] view of a flat [128, X] tile AP at column offset/stride
        return bass.AP(tensor=ap2d.tensor, offset=ap2d.offset + off,
                       ap=[list(ap2d.ap[0]), [stride, count]])

    def bcast2(ap2d, ncols):
        # [128, 2, ncols] view of a [128, ncols] tile, broadcast on mid dim
        return bass.AP(tensor=ap2d.tensor, offset=ap2d.offset,
                       ap=[list(ap2d.ap[0]), [0, 2], [1, ncols]])

    def win3(ap2d, th_unused=None):
        # [128, 2, 1024] data view of a flat [128, 2050] window tile
        return ap2d[:, 0:2050].rearrange("p (a l) -> p a l", a=2)[:, :, 0:HL]

    with tile.TileContext(nc) as tc, contextlib.ExitStack() as ctx:
        consts = ctx.enter_context(tc.tile_pool(name="consts", bufs=1))
        wtmp = ctx.enter_context(tc.tile_pool(name="wtmp", bufs=1))
        big = ctx.enter_context(tc.tile_pool(name="big", bufs=1))
        sm = ctx.enter_context(tc.tile_pool(name="sm", bufs=4))
        pa = ctx.enter_context(tc.tile_pool(name="pa", bufs=2))
        pbz = ctx.enter_context(tc.tile_pool(name="pbz", bufs=4))
        pbc = ctx.enter_context(tc.tile_pool(name="pbc", bufs=3))
        pw8 = ctx.enter_context(tc.tile_pool(name="pw8", bufs=1))
        ps = ctx.enter_context(tc.tile_pool(name="ps", bufs=2, space="PSUM"))
        psy = ctx.enter_context(tc.tile_pool(name="psy", bufs=1, space="PSUM"))

        def load_const(name, prm, shape, dtype, pool=consts):
            t = pool.tile(shape, dtype, tag=name, name=name)
            nc.sync.dma_start(out=t[:], in_=prm.ap())
            return t

        c_ident_bf = load_const("c_ident_bf", ident_bf, [128, 128], BF)
        c_ones_fr = load_const("c_ones_fr", ones_f32r, [128, 1], FR)
        c_ones_bf = load_const("c_ones_bf", ones_bf, [128, 1], BF)
        c_xproj = load_const("c_xproj", xproj_lhsT, [128, DT, 96], FR)
        c_dtproj = load_const("c_dtproj", dtproj_lhsT, [R, DI], FR)
        c_outproj = load_const("c_outproj", outproj_lhsT, [128, DT, D], BF)
        c_A = load_const("c_A", A_cols, [128, DT, N], F)
        c_lconv_b = load_const("c_lconv_b", lconv_b, [128, DD, 1], F)
        c_lnc_w = load_const("c_lnc_w", lnc_w, [128, DD, 1], F)
        c_lnc_b = load_const("c_lnc_b", lnc_b, [128, DD, 1], F)
        c_mconv_b = load_const("c_mconv_b", mconv_b, [128, DT, 1], F)
        c_dtproj_b = load_const("c_dtproj_b", dtproj_b, [128, DT, 1], F)
        c_ssmD = load_const("c_ssmD", ssm_D, [128, DT, 1], F)
        c_lnp_w = load_const("c_lnp_w", lnp_w, [128, 2 * DD, 1], F)
        c_lnp_b = load_const("c_lnp_b", lnp_b, [128, 2 * DD, 1], F)
        c_pconv_b = load_const("c_pconv_b", pconv_b, [128, DD, 1], F)
        c_mlp1_b = load_const("c_mlp1_b", mlp1_b, [128, 8, 1], F)
        c_mlp2_b = load_const("c_mlp2_b", mlp2_b, [128, DD, 1], F)
        c_w0 = load_const("c_w0", w0_col, [128, 1], F)
        c_rmsw = consts.tile([128, D], F, tag="c_rmsw", name="c_rmsw")
        nc.sync.dma_start(out=c_rmsw[:], in_=brow(rms_w_row, 0, D))
        eps6 = consts.tile([128, 1], F, tag="eps6", name="eps6")
        nc.vector.memset(eps6[:], 1e-6)
        eps5 = consts.tile([128, 1], F, tag="eps5", name="eps5")
        nc.vector.memset(eps5[:], 1e-5)

        def wload(prm, shape, dtype, nm, tag="wbig"):
            t = wtmp.tile(shape, dtype, tag=tag, name=nm)
            nc.sync.dma_start(out=t[:], in_=prm.ap())
            return t

        xnT_pad = big.tile([128, DD, L + 2], BF, tag="xnT8", name="xnT_pad")
        sz = big.tile([128, DT, L], BF, tag="sz16", name="sz")
        hedge = big.tile([128, 2 * N, 2], F, tag="hedge", name="hedge")

        # manual ring buffers for the scan phase
        NAW = 2
        NBW = 2
        NHW = 2
        aw_ring = [big.tile([128, 2050], BF, tag=f"aw{j}", name=f"aw{j}")
                   for j in range(NAW)]
        bw_ring = [big.tile([128, 2050], BF, tag=f"bw{j}", name=f"bw{j}")
                   for j in range(NBW)]
        hw_ring = [big.tile([128, 2050], BF, tag=f"hw{j}", name=f"hw{j}")
                   for j in range(NHW)]
        for j in range(NAW):
            nc.vector.memset(aw_ring[j][:, HL:HL + 1], 0.0)
            nc.vector.memset(aw_ring[j][:, 2049:2050], 0.0)

        # ---- A1+A2: rmsnorm fused with transpose ---------------------------
        nc.vector.memset(xnT_pad[:, :, 0:1], 0.0)
        nc.vector.memset(xnT_pad[:, :, L + 1:L + 2], 0.0)
        for tt in range(NT):
            x_t = pa.tile([128, 512], F, tag="a_t", name="x_t")
            nc.gpsimd.dma_start(out=x_t[:, :D], in_=x_in.ap()[tt * 128:(tt + 1) * 128, :])
            sq = pbz.tile([128, 512], BF, tag="sq2", name="sq", bufs=1)
            ssq = sm.tile([128, 1], F, tag="ssq", name="ssq")
            nc.scalar.activation(sq[:, :D], x_t[:, :D], AF.Square, accum_out=ssq[:])
            rsq = sm.tile([128, 1], F, tag="rsq", name="rsq")
            nc.scalar.activation(rsq[:], ssq[:], AF.Sqrt, bias=eps6[:], scale=1.0 / D)
            nc.vector.reciprocal(rsq[:], rsq[:])
            xn_t = pbz.tile([128, 512], BF, tag="xn_t", name="xn_t", bufs=1)
            nc.scalar.activation(xn_t[:, :D], x_t[:, :D], AF.Copy, scale=rsq[:])
            nc.vector.tensor_tensor(out=xn_t[:, :D], in0=xn_t[:, :D], in1=c_rmsw[:], op=MUL)
            for dd in range(DD):
                pt = ps.tile([128, 512], BF, tag="convB", name="pt", bufs=1)
                nc.tensor.transpose(pt[:, :128], xn_t[:, dd * 128:(dd + 1) * 128], c_ident_bf[:])
                nc.scalar.copy(xnT_pad[:, dd, 1 + tt * 128:1 + (tt + 1) * 128], pt[:, :128])

        # ---- A3: local conv + square --------------------------------------
        c_lconv = wload(lconv_diag, [128, 3 * DD, 128], BF, "c_lconv")
        xc = big.tile([128, DD, L], BF, tag="beta16x", name="xc")
        xc2 = big.tile([128, DD, L], BF, tag="xc2mix", name="xc2")
        for dd in range(DD):
            for c in range(CH):
                pc = ps.tile([128, 512], F, tag="convA", name="pc")
                for k in range(3):
                    nc.tensor.matmul(pc[:], c_lconv[:, k * DD + dd, :],
                                     xnT_pad[:, dd, k + c * 512:k + c * 512 + 512],
                                     start=(k == 0), stop=(k == 2))
                nc.scalar.activation(xc[:, dd, c * 512:(c + 1) * 512], pc[:], AF.Identity,
                                     bias=c_lconv_b[:, dd, :])
                nc.scalar.activation(xc2[:, dd, c * 512:(c + 1) * 512], pc[:], AF.Square,
                                     bias=c_lconv_b[:, dd, :])

        # ---- A4: LNc stats + apply ----------------------------------------
        for si, src in ((0, xc), (1, xc2)):
            for c in range(CH):
                pst = ps.tile([1, 512], F, tag="stat", name="pst", bufs=1)
                for dd in range(DD):
                    nc.tensor.matmul(pst[:], c_ones_bf[:],
                                     src[:, dd, c * 512:(c + 1) * 512],
                                     start=(dd == 0), stop=(dd == DD - 1))
                sev = sm.tile([1, 512], F, tag="sev", name="sev")
                nc.vector.tensor_copy(out=sev[:], in_=pst[:])
                nc.sync.dma_start(out=stat_dram.ap()[si, c * 512:(c + 1) * 512].unsqueeze(0),
                                  in_=sev[:])
        st = sm.tile([128, 2, 16], F, tag="st", name="st")
        nc.sync.dma_start(out=st[:], in_=stat_dram.ap()[0:2, :].rearrange("s (p i) -> p s i", p=128))
        mu = sm.tile([128, 16], F, tag="mu", name="mu")
        nc.vector.tensor_scalar_mul(mu[:], st[:, 0, :], 1.0 / D)
        var = sm.tile([128, 16], F, tag="var", name="var")
        nc.vector.tensor_tensor(out=var[:], in0=mu[:], in1=mu[:], op=MUL)
        qv = sm.tile([128, 16], F, tag="qv", name="qv")
        nc.vector.tensor_scalar_mul(qv[:], st[:, 1, :], 1.0 / D)
        nc.vector.tensor_tensor(out=var[:], in0=qv[:], in1=var[:], op=SUB)
        rstd = sm.tile([128, 16], F, tag="rstd", name="rstd")
        nc.scalar.activation(rstd[:], var[:], AF.Sqrt, bias=eps5[:])
        nc.vector.reciprocal(rstd[:], rstd[:])
        nu = sm.tile([128, 16], F, tag="nu", name="nu")
        nc.vector.tensor_tensor(out=nu[:], in0=mu[:], in1=rstd[:], op=MUL)
        nc.sync.dma_start(out=stat_dram.ap()[2, :].rearrange("(p i) -> p i", p=128), in_=rstd[:])
        nc.sync.dma_start(out=stat_dram.ap()[3, :].rearrange("(p i) -> p i", p=128), in_=nu[:])
        xcsT = big.tile([128, DD, L], BF, tag="delta16x", name="xcsT")
        for hh in range(2):
            rs_bc = pw8.tile([128, HL], BF, tag="t8w", name="rs_bc")
            nc.gpsimd.dma_start(out=rs_bc[:], in_=brow(stat_dram, 2 * L + hh * HL, HL))
            nu_bc = pw8.tile([128, HL], BF, tag="t8x", name="nu_bc")
            nc.gpsimd.dma_start(out=nu_bc[:], in_=brow(stat_dram, 3 * L + hh * HL, HL))
            for dd in range(DD):
                t1 = pw8.tile([128, HL], BF, tag="t8y", name="t1")
                hsl = slice(hh * HL, (hh + 1) * HL)
                nc.vector.tensor_tensor(out=t1[:], in0=xc[:, dd, hsl], in1=rs_bc[:], op=MUL)
                nc.vector.tensor_tensor(out=t1[:], in0=t1[:], in1=nu_bc[:], op=SUB)
                nc.scalar.activation(xcsT[:, dd, hsl], t1[:], AF.Silu, bias=c_lnc_b[:, dd, :],
                                     scale=c_lnc_w[:, dd, :])

        # ---- A5: in_proj ----------------------------------------------------
        c_inproj = wload(inproj_lhsT, [128, DD, 2 * DI], BF, "c_inproj")
        xmT_pad = big.tile([128, DT, L + 6], BF, tag="g16", name="xmT_pad")
        nc.vector.memset(xmT_pad[:, :, 0:3], 0.0)
        nc.vector.memset(xmT_pad[:, :, L + 3:L + 6], 0.0)
        for m in range(8):
            for c in range(CH):
                pc = ps.tile([128, 512], F, tag="convA", name="pc2")
                for kk in range(DD):
                    nc.tensor.matmul(pc[:], c_inproj[:, kk, m * 128:(m + 1) * 128],
                                     xcsT[:, kk, c * 512:(c + 1) * 512],
                                     start=(kk == 0), stop=(kk == DD - 1))
                if m < 4:
                    nc.scalar.copy(xmT_pad[:, m, 3 + c * 512:3 + (c + 1) * 512], pc[:])
                else:
                    nc.scalar.activation(sz[:, m - 4, c * 512:(c + 1) * 512], pc[:], AF.Silu)

        # ---- A6(d): mamba conv + x_proj + dt/beta/xmd ----------------------
        c_mconv = wload(mconv_diag, [128, 4 * DT, 128], BF, "c_mconv2", tag="wbig2")

        def stage_a6(d):
            xm_bf = big.tile([128, DT, L], BF, tag="xs16", name=f"xm_bf_{d}")
            dtT = big.tile([R, L], FR, tag="dtT", name=f"dtT_{d}")
            for c in range(CH):
                xm_fr = pa.tile([128, DT, 512], FR, tag="xm_fr", name="xm_fr", bufs=1)
                for dt in range(DT):
                    pc = ps.tile([128, 512], F, tag="convA", name="pc3")
                    for k in range(4):
                        off = (k if d == "f" else 6 - k) + c * 512
                        nc.tensor.matmul(pc[:], c_mconv[:, k * DT + dt, :],
                                         xmT_pad[:, dt, off:off + 512],
                                         start=(k == 0), stop=(k == 3))
                    nc.scalar.activation(xm_fr[:, dt, :], pc[:], AF.Silu, bias=c_mconv_b[:, dt, :])
                    nc.scalar.activation(xm_bf[:, dt, c * 512:(c + 1) * 512], pc[:], AF.Silu,
                                         bias=c_mconv_b[:, dt, :])
                psx = ps.tile([96, 512], F, tag="stat", name="psx", bufs=1)
                for dt in range(DT):
                    nc.tensor.matmul(psx[:], c_xproj[:, dt, :], xm_fr[:, dt, :],
                                     start=(dt == 0), stop=(dt == DT - 1))
                nc.scalar.copy(dtT[:, c * 512:(c + 1) * 512], psx[0:R, :])
                bc_ev = pbz.tile([64, 512], BF, tag="bc_ev", name="bc_ev", bufs=1)
                nc.scalar.copy(bc_ev[0:32, :], psx[32:64, :])
                nc.scalar.copy(bc_ev[32:64, :], psx[64:96, :])
                nc.sync.dma_start(out=bc_dram[d].ap()[:, c * 512:(c + 1) * 512], in_=bc_ev[:])
            drow = 0 if d == "f" else 2
            for ms, cs in (((0, 1), (0, 1)), ((0, 1), (2, 3)), ((2, 3), (0, 1, 2, 3))):
                e1t = big.tile([128, 2, L], BF, tag="xc2mix", name=f"e1t_{d}")
                for mi, m in enumerate(ms):
                    for c in cs:
                        pc = ps.tile([128, 512], F, tag="convA", name="pc4")
                        nc.tensor.matmul(pc[:], c_dtproj[:, m * 128:(m + 1) * 128],
                                         dtT[:, c * 512:(c + 1) * 512], start=True, stop=True)
                        nc.scalar.activation(e1t[:, mi, c * 512:(c + 1) * 512], pc[:], AF.Exp,
                                             bias=c_dtproj_b[:, m, :])
                for mi, m in enumerate(ms):
                    for c in cs:
                        dev = pbz.tile([128, 512], BF, tag="dev", name="dev", bufs=2)
                        nc.scalar.activation(dev[:], e1t[:, mi, c * 512:(c + 1) * 512], AF.Ln,
                                             bias=1.0)
                        nc.sync.dma_start(
                            out=db_dram.ap()[drow, :, m * L + c * 512:m * L + (c + 1) * 512],
                            in_=dev[:])
                        bev = pbz.tile([128, 512], BF, tag="bev", name="bev", bufs=2)
                        nc.vector.tensor_tensor(out=bev[:], in0=dev[:],
                                                in1=xm_bf[:, m, c * 512:(c + 1) * 512], op=MUL)
                        nc.sync.dma_start(
                            out=db_dram.ap()[drow + 1, :, m * L + c * 512:m * L + (c + 1) * 512],
                            in_=bev[:])
            for dt in range(DT):
                for c in range(CH):
                    xev = pbz.tile([128, 512], BF, tag="xev", name="xev", bufs=2)
                    nc.scalar.activation(xev[:], xm_bf[:, dt, c * 512:(c + 1) * 512],
                                         AF.Copy, scale=c_ssmD[:, dt, :])
                    nc.sync.dma_start(
                        out=xmd_dram[d].ap()[:, dt * L + c * 512:dt * L + (c + 1) * 512],
                        in_=xev[:])

        # ---- scan phase: one (direction, pair) window group ----------------
        gtiles = {}
        ring_idx = [0, 0, 0]  # aw, bw, hw

        def stage_scan_pair(d, pair):
            drow = 0 if d == "f" else 2
            if pair == 0:
                tag = "gdir" if d == "f" else "g16"
                gtiles[d] = big.tile([128, DT, L], BF, tag=tag, name=f"g_{d}")
            g = gtiles[d]
            delta_p = big.tile([128, 2, L], BF, tag="delta16x", name=f"delta_{d}{pair}")
            beta_p = big.tile([128, 2, L], BF, tag="beta16x", name=f"beta_{d}{pair}")
            ths = (0, 1) if d == "f" else (1, 0)
            for sh in (ths[0], ths[1]):
                hs = slice(sh * HL, (sh + 1) * HL)
                nc.sync.dma_start(out=delta_p[:, :, hs], in_=db_dram.ap()[
                    drow, :, 2 * pair * L:(2 * pair + 2) * L].rearrange(
                        "p (a l) -> p a l", a=2)[:, :, hs])
                nc.sync.dma_start(out=beta_p[:, :, hs], in_=db_dram.ap()[
                    drow + 1, :, 2 * pair * L:(2 * pair + 2) * L].rearrange(
                        "p (a l) -> p a l", a=2)[:, :, hs])
            for thi, th in enumerate(ths):
                # prezero bw link columns at window start (first th only needs
                # 0-link; second th overwrites the link with the carried state)
                if thi == 0:
                    for j in range(NBW):
                        nc.vector.memset(bw_ring[j][:, HL:HL + 1], 0.0)
                        nc.vector.memset(bw_ring[j][:, 2049:2050], 0.0)
                yp = psy.tile([128, 2, HL], F, tag="ypair", name="yp")
                xmd_t = {}
                for i in range(2):
                    dt = pair * 2 + i
                    xmd_t[i] = pbz.tile([128, HL], BF, tag=f"xmd{i}", name="xmd_c", bufs=1)
                    nc.gpsimd.dma_start(
                        out=xmd_t[i][:],
                        in_=xmd_dram[d].ap()[:, dt * L + th * HL:dt * L + (th + 1) * HL])
                for n in range(N):
                    slot = pair * N + n
                    B_bc = pbc.tile([128, HL], BF, tag="B_bc", name="B_bc", bufs=2)
                    nc.gpsimd.dma_start(out=B_bc[:], in_=brow(bc_dram[d], n * L + th * HL, HL))
                    C_bc = pbc.tile([128, HL], BF, tag="C_bc", name="C_bc", bufs=2)
                    nc.gpsimd.dma_start(out=C_bc[:],
                                        in_=brow(bc_dram[d], (N + n) * L + th * HL, HL))
                    aw = aw_ring[ring_idx[0] % NAW]; ring_idx[0] += 1
                    bw = bw_ring[ring_idx[1] % NBW]; ring_idx[1] += 1
                    hw = hw_ring[ring_idx[2] % NHW]; ring_idx[2] += 1
                    r_aw = win3(aw[:])
                    for i in range(2):
                        nc.scalar.activation(r_aw[:, i, :], delta_p[:, i, th * HL:(th + 1) * HL],
                                             AF.Exp, scale=c_A[:, pair * 2 + i, n:n + 1])
                    nc.vector.tensor_tensor(out=win3(bw[:]),
                                            in0=beta_p[:, :, th * HL:(th + 1) * HL],
                                            in1=bcast2(B_bc[:], HL), op=MUL)
                    if thi == 1:
                        src = hedge[:, slot, 1:2] if d == "f" else hedge[:, slot, 0:1]
                        nc.vector.tensor_copy(out=bw[:, HL:HL + 1], in_=src)
                        init = hedge[:, slot, 0:1] if d == "f" else hedge[:, slot, 1:2]
                    else:
                        init = 0.0
                    if d == "f":
                        nc.vector.tensor_tensor_scan(hw[:, 0:W2], aw[:, 0:W2], bw[:, 0:W2],
                                                     init, MUL, ADD)
                    else:
                        nc.vector.tensor_tensor_scan(hw[:, 0:W2][:, ::-1], aw[:, 0:W2][:, ::-1],
                                                     bw[:, 0:W2][:, ::-1], init, MUL, ADD)
                    if thi == 0:
                        off = 1023 if d == "f" else 0
                        nc.vector.tensor_copy(out=hedge[:, slot, 0:2],
                                              in_=strided(hw[:], off, 1025, 2))
                    nc.vector.tensor_tensor(out=win3(hw[:]), in0=win3(hw[:]),
                                            in1=bcast2(C_bc[:], HL), op=MUL)
                    for i in range(2):
                        for c2 in range(2):
                            base = i * 1025 + c2 * 512
                            nc.tensor.matmul(yp[:, i, c2 * 512:(c2 + 1) * 512], c_ident_bf[:],
                                             hw[:, base:base + 512],
                                             start=(n == 0), stop=False)
                for i in range(2):
                    for c2 in range(2):
                        nc.tensor.matmul(yp[:, i, c2 * 512:(c2 + 1) * 512], c_ident_bf[:],
                                         xmd_t[i][:, c2 * 512:(c2 + 1) * 512],
                                         start=False, stop=True)
                nc.vector.tensor_tensor(
                    out=g[:, 2 * pair:2 * pair + 2, th * HL:(th + 1) * HL],
                    in0=yp[:, :, :],
                    in1=sz[:, 2 * pair:2 * pair + 2, th * HL:(th + 1) * HL], op=MUL)

        # ---- C/E: out_proj + residual --------------------------------------
        def stage_outproj(d, xs):
            g = gtiles[d]
            base = 0 if d == "f" else DD
            for m in range(DD):
                for c in range(CH):
                    pc = ps.tile([128, 512], F, tag="convA", name="pc5")
                    for dt in range(DT):
                        nc.tensor.matmul(pc[:], c_outproj[:, dt, m * 128:(m + 1) * 128],
                                         g[:, dt, c * 512:(c + 1) * 512],
                                         start=(dt == 0), stop=False)
                    nc.tensor.matmul(pc[:], c_ident_bf[:],
                                     xnT_pad[:, m, 1 + c * 512:1 + (c + 1) * 512],
                                     start=False, stop=True)
                    nc.scalar.copy(xs[:, base + m, c * 512:(c + 1) * 512], pc[:])

        # ---- F3: MLP -> mlpT rows in DRAM ----------------------------------
        def stage_mlp():
            c_mlp1 = wload(mlp1_lhsT, [128, DD, 4 * D], BF, "c_mlp1")
            c_mlp2 = wload(mlp2_lhsT, [128, 8, D], BF, "c_mlp2", tag="wbig2")
            for c in range(CH):
                h1 = big.tile([128, 8, 512], BF, tag="xc2mix", name="h1")
                for m in range(8):
                    pc = ps.tile([128, 512], F, tag="convA", name="pc7")
                    for kk in range(DD):
                        nc.tensor.matmul(pc[:], c_mlp1[:, kk, m * 128:(m + 1) * 128],
                                         xnT_pad[:, kk, 1 + c * 512:1 + (c + 1) * 512],
                                         start=(kk == 0), stop=(kk == DD - 1))
                    nc.scalar.activation(h1[:, m, :], pc[:], AF.Gelu, bias=c_mlp1_b[:, m, :])
                mlpc = pbz.tile([128, DD, 512], BF, tag="b_t", name="mlpc", bufs=2)
                for m2 in range(DD):
                    pc = ps.tile([128, 512], F, tag="convA", name="pc8")
                    for mk in range(8):
                        nc.tensor.matmul(pc[:], c_mlp2[:, mk, m2 * 128:(m2 + 1) * 128],
                                         h1[:, mk, :], start=(mk == 0), stop=(mk == 7))
                    nc.scalar.activation(mlpc[:, m2, :], pc[:], AF.Identity,
                                         bias=c_mlp2_b[:, m2, :])
                for q in range(4):
                    tt = c * 4 + q
                    mt = pbz.tile([128, 512], BF, tag="z_t", name="mt", bufs=2)
                    for m2 in range(DD):
                        pt3 = ps.tile([128, 512], BF, tag="convB", name="pt3", bufs=1)
                        nc.tensor.transpose(pt3[:, :128], mlpc[:, m2, q * 128:(q + 1) * 128],
                                            c_ident_bf[:])
                        nc.scalar.copy(mt[:, m2 * 128:(m2 + 1) * 128], pt3[:, :128])
                    nc.sync.dma_start(out=mlpT_dram.ap()[tt * 128:(tt + 1) * 128, :],
                                      in_=mt[:, :D])

        # ================= emission schedule =================
        stage_a6("f")
        stage_scan_pair("f", 0)
        stage_a6("b")
        xs = big.tile([128, 2 * DD, L], BF, tag="xs16", name="xs")
        stage_scan_pair("f", 1)
        stage_outproj("f", xs)
        stage_scan_pair("b", 0)
        stage_mlp()
        stage_scan_pair("b", 1)
        stage_outproj("b", xs)
        if DBG:
            nc.sync.dma_start(out=dbg_sz.ap(), in_=sz[:])
            nc.sync.dma_start(out=dbg_gf.ap(), in_=gtiles["f"][:])
            nc.sync.dma_start(out=dbg_gb.ap(), in_=gtiles["b"][:])
            nc.sync.dma_start(out=dbg_xs.ap(), in_=xs[:])

        # ---- F1: LNp ---------------------------------------------------------
        xs2 = big.tile([128, 2 * DD, L], BF, tag="gdir", name="xs2")
        for dt in range(2 * DD):
            nc.scalar.activation(xs2[:, dt, :], xs[:, dt, :], AF.Square)
        for si, src in ((4, xs), (5, xs2)):
            for c in range(CH):
                pst = ps.tile([1, 512], F, tag="stat", name="pst2", bufs=1)
                for dt in range(2 * DD):
                    nc.tensor.matmul(pst[:], c_ones_bf[:], src[:, dt, c * 512:(c + 1) * 512],
                                     start=(dt == 0), stop=(dt == 2 * DD - 1))
                sev2 = sm.tile([1, 512], F, tag="sev", name="sev2")
                nc.vector.tensor_copy(out=sev2[:], in_=pst[:])
                nc.sync.dma_start(out=stat_dram.ap()[si, c * 512:(c + 1) * 512].unsqueeze(0),
                                  in_=sev2[:])
        stp = sm.tile([128, 2, 16], F, tag="st", name="stp")
        nc.sync.dma_start(out=stp[:], in_=stat_dram.ap()[4:6, :].rearrange("s (p i) -> p s i", p=128))
        mup = sm.tile([128, 16], F, tag="mu", name="mup")
        nc.vector.tensor_scalar_mul(mup[:], stp[:, 0, :], 1.0 / (2 * D))
        varp = sm.tile([128, 16], F, tag="var", name="varp")
        nc.vector.tensor_tensor(out=varp[:], in0=mup[:], in1=mup[:], op=MUL)
        qp = sm.tile([128, 16], F, tag="qv", name="qp")
        nc.vector.tensor_scalar_mul(qp[:], stp[:, 1, :], 1.0 / (2 * D))
        nc.vector.tensor_tensor(out=varp[:], in0=qp[:], in1=varp[:], op=SUB)
        rstdp = sm.tile([128, 16], F, tag="rstd", name="rstdp")
        nc.scalar.activation(rstdp[:], varp[:], AF.Sqrt, bias=eps5[:])
        nc.vector.reciprocal(rstdp[:], rstdp[:])
        nup = sm.tile([128, 16], F, tag="nu", name="nup")
        nc.vector.tensor_tensor(out=nup[:], in0=mup[:], in1=rstdp[:], op=MUL)
        nc.sync.dma_start(out=stat_dram.ap()[6, :].rearrange("(p i) -> p i", p=128), in_=rstdp[:])
        nc.sync.dma_start(out=stat_dram.ap()[7, :].rearrange("(p i) -> p i", p=128), in_=nup[:])
        xsn_pad = big.tile([128, 2 * DD, L + 2], BF, tag="g16", name="xsn_pad")
        nc.vector.memset(xsn_pad[:, :, 0:1], 0.0)
        nc.vector.memset(xsn_pad[:, :, L + 1:L + 2], 0.0)
        for hh in range(2):
            rsp_bc = pw8.tile([128, HL], BF, tag="t8w", name="rsp_bc")
            nc.gpsimd.dma_start(out=rsp_bc[:], in_=brow(stat_dram, 6 * L + hh * HL, HL))
            nup_bc = pw8.tile([128, HL], BF, tag="t8x", name="nup_bc")
            nc.gpsimd.dma_start(out=nup_bc[:], in_=brow(stat_dram, 7 * L + hh * HL, HL))
            for dt in range(2 * DD):
                t1 = pw8.tile([128, HL], BF, tag="t8y", name="t1p")
                hsl = slice(hh * HL, (hh + 1) * HL)
                nc.vector.tensor_tensor(out=t1[:], in0=xs[:, dt, hsl], in1=rsp_bc[:], op=MUL)
                nc.vector.tensor_tensor(out=t1[:], in0=t1[:], in1=nup_bc[:], op=SUB)
                nc.scalar.activation(xsn_pad[:, dt, 1 + hh * HL:1 + (hh + 1) * HL], t1[:],
                                     AF.Identity, bias=c_lnp_b[:, dt, :], scale=c_lnp_w[:, dt, :])

        # ---- F2: post conv ---------------------------------------------------
        c_psel = wload(pconv_sel, [128, DD * 2 * 3, 128], BF, "c_psel")
        mixer = big.tile([128, DD, L], BF, tag="xc2mix", name="mixer")
        for gt in range(DD):
            for c in range(CH):
                pc = ps.tile([128, 512], F, tag="convA", name="pc6")
                first = True
                for jj in range(2):
                    ct = 2 * gt + jj
                    for k in range(3):
                        nc.tensor.matmul(pc[:], c_psel[:, (gt * 2 + jj) * 3 + k, :],
                                         xsn_pad[:, ct, k + c * 512:k + c * 512 + 512],
                                         start=first, stop=(jj == 1 and k == 2))
                        first = False
                nc.scalar.activation(mixer[:, gt, c * 512:(c + 1) * 512], pc[:], AF.Silu,
                                     bias=c_pconv_b[:, gt, :])

        # ---- F4: transpose back + combine + store ----------------------------
        for tt in range(NT):
            mixT = pbz.tile([128, 256], BF, tag="mxT", name="mixT", bufs=3)
            mlpT = pbz.tile([128, 256], BF, tag="mpT", name="mlpT", bufs=3)
            nc.sync.dma_start(out=mlpT[:], in_=mlpT_dram.ap()[tt * 128:(tt + 1) * 128, :])
            for dd in range(DD):
                pt = ps.tile([128, 512], BF, tag="convA", name="ptb")
                nc.tensor.transpose(pt[:, :128], mixer[:, dd, tt * 128:(tt + 1) * 128],
                                    c_ident_bf[:])
                nc.scalar.copy(mixT[:, dd * 128:(dd + 1) * 128], pt[:, :128])
            x_t2 = pa.tile([128, 512], F, tag="a_t", name="x_t2")
            nc.gpsimd.dma_start(out=x_t2[:, :D], in_=x_in.ap()[tt * 128:(tt + 1) * 128, :])
            o_t = pbz.tile([128, 256], F, tag="o_t", name="o_t", bufs=2)
            nc.vector.tensor_scalar_mul(o_t[:], mixT[:], c_w0[:])
            nc.vector.tensor_tensor(out=o_t[:], in0=o_t[:], in1=x_t2[:, :D], op=ADD)
            nc.vector.tensor_tensor(out=o_t[:], in0=o_t[:], in1=mlpT[:], op=ADD)
            nc.sync.dma_start(out=out_dram.ap()[tt * 128:(tt + 1) * 128, :], in_=o_t[:])

    nc.compile()
    return nc


def kernel(**inputs):
    global _BUILT
    from concourse.bass_utils import run_bass_kernel_spmd

    if _BUILT is None:
        _BUILT = _build()
    nc = _BUILT

    w = _host_prep(inputs)
    x = np.asarray(inputs["x"], dtype=np.float32)
    in_maps = []
    for b in range(B):
        m = dict(w)
        m["x"] = np.ascontiguousarray(x[b])
        in_maps.append(m)
    res = run_bass_kernel_spmd(nc, in_maps, list(range(B)))
    out = np.stack([res.results[b]["out"] for b in range(B)], axis=0)
    return out.astype(np.float32)


if __name__ == "__main__":
    rng = np.random.default_rng(0)
    fake = {
        "x": rng.standard_normal((B, L, D), dtype=np.float32),
        "rms_w": np.ones(D, np.float32),
        "local_conv_w": rng.standard_normal((D, 1, 3), dtype=np.float32) * 0.3,
        "local_conv_b": np.zeros(D, np.float32),
        "lnc_w": np.ones(D, np.float32),
        "lnc_b": np.zeros(D, np.float32),
        "in_proj_w": rng.standard_normal((2 * DI, D), dtype=np.float32) * 0.02,
        "conv1d_w": rng.standard_normal((DI, 1, 4), dtype=np.float32) * 0.3,
        "conv1d_b": np.zeros(DI, np.float32),
        "x_proj_w": rng.standard_normal((R + 2 * N, DI), dtype=np.float32) * 0.02,
        "dt_proj_w": rng.standard_normal((DI, R), dtype=np.float32) * 0.1,
        "dt_proj_b": np.full(DI, -4.0, np.float32),
        "A_log": np.log(np.tile(np.arange(1, N + 1, dtype=np.float32), (DI, 1))),
        "ssm_D": np.ones(DI, np.float32),
        "out_proj_w": rng.standard_normal((D, DI), dtype=np.float32) * 0.02,
        "lnp_w": np.ones(2 * D, np.float32),
        "lnp_b": np.zeros(2 * D, np.float32),
        "post_conv_w": rng.standard_normal((D, 2, 3), dtype=np.float32) * 0.3,
        "post_conv_b": np.zeros(D, np.float32),
        "mlp_w1": rng.standard_normal((4 * D, D), dtype=np.float32) * 0.02,
        "mlp_b1": np.zeros(4 * D, np.float32),
        "mlp_w2": rng.standard_normal((D, 4 * D), dtype=np.float32) * 0.02,
        "mlp_b2": np.zeros(D, np.float32),
        "branch_logits": np.array([1.0, 0.1], np.float32),
    }
    out = kernel(**fake)
    print("kernel ran, out shape", out.shape, "finite:", bool(np.isfinite(out).all()))


# revision 24
# speedup vs baseline: 1.0307x; 1.0085x over previous
"""Bidirectional Conv-Mamba block for Trainium2, 8-core batch-parallel.

kernel(**inputs) takes FULL unsharded inputs, shards batch over 8 NeuronCores
(1 batch element per core), returns the FULL [8, 2048, 256] float32 output.
"""

import os
import sys

for _p in ("/opt/trn_rl_repo", "/root/.axon_site/_ro/trn_rl_repo"):
    if os.path.isdir(_p) and _p not in sys.path:
        sys.path.append(_p)

import numpy as np
import ml_dtypes

B = 8
L = 2048
HL = 1024        # time half
D = 256
DI = 512
N = 32
R = 16
NT = L // 128
DD = D // 128    # 2
DT = DI // 128   # 4
CH = L // 512    # 4
W2 = 2 * HL + 1  # scanned columns per window

_BUILT = None


def _bf(x):
    return np.ascontiguousarray(x.astype(ml_dtypes.bfloat16))


def _f32(x):
    return np.ascontiguousarray(x.astype(np.float32))


def _pack_cols(v, ntiles):
    k = v.shape[1]
    return np.ascontiguousarray(v.reshape(ntiles, 128, k).transpose(1, 0, 2))


def _host_prep(inputs):
    p = {k: np.asarray(v, dtype=np.float32) for k, v in inputs.items()}
    out = {}
    out["rms_w_row"] = _f32(p["rms_w"].reshape(1, D))

    lconv = np.zeros((128, 3 * DD, 128), np.float32)
    for k in range(3):
        for dd in range(DD):
            lconv[:, k * DD + dd, :][np.arange(128), np.arange(128)] = \
                p["local_conv_w"][dd * 128:(dd + 1) * 128, 0, k]
    out["lconv_diag"] = _bf(lconv)
    out["lconv_b"] = _f32(_pack_cols(p["local_conv_b"].reshape(D, 1), DD))
    out["lnc_w"] = _f32(_pack_cols(p["lnc_w"].reshape(D, 1), DD))
    out["lnc_b"] = _f32(_pack_cols(p["lnc_b"].reshape(D, 1), DD))

    out["inproj_lhsT"] = _bf(_pack_cols(p["in_proj_w"].T, DD))

    mconv = np.zeros((128, 4 * DT, 128), np.float32)
    for k in range(4):
        for dt in range(DT):
            mconv[:, k * DT + dt, :][np.arange(128), np.arange(128)] = \
                p["conv1d_w"][dt * 128:(dt + 1) * 128, 0, k]
    out["mconv_diag"] = _bf(mconv)
    out["mconv_b"] = _f32(_pack_cols(p["conv1d_b"].reshape(DI, 1), DT))

    xp = np.zeros((DI, 96), np.float32)
    xp[:, 0:R] = p["x_proj_w"].T[:, 0:R]
    xp[:, 32:96] = p["x_proj_w"].T[:, R:80]
    out["xproj_lhsT"] = _f32(_pack_cols(xp, DT))
    out["dtproj_lhsT"] = _f32(p["dt_proj_w"].T)
    out["dtproj_b"] = _f32(_pack_cols(p["dt_proj_b"].reshape(DI, 1), DT))
    out["A_cols"] = _f32(_pack_cols(-np.exp(p["A_log"]), DT))
    out["ssm_D"] = _f32(_pack_cols(p["ssm_D"].reshape(DI, 1), DT))
    out["outproj_lhsT"] = _bf(_pack_cols(p["out_proj_w"].T, DT))
    out["lnp_w"] = _f32(_pack_cols(p["lnp_w"].reshape(2 * D, 1), 2 * DD))
    out["lnp_b"] = _f32(_pack_cols(p["lnp_b"].reshape(2 * D, 1), 2 * DD))

    psel = np.zeros((128, DD * 2 * 3, 128), np.float32)
    for g in range(D):
        gt, gl = divmod(g, 128)
        for j in range(2):
            c = 2 * g + j
            ct, cl = divmod(c, 128)
            jj = ct - 2 * gt
            for k in range(3):
                psel[cl, (gt * 2 + jj) * 3 + k, gl] = p["post_conv_w"][g, j, k]
    out["pconv_sel"] = _bf(psel)
    out["pconv_b"] = _f32(_pack_cols(p["post_conv_b"].reshape(D, 1), DD))

    out["mlp1_lhsT"] = _bf(_pack_cols(p["mlp_w1"].T, DD))
    out["mlp1_b"] = _f32(_pack_cols(p["mlp_b1"].reshape(4 * D, 1), 8))
    w = np.exp(p["branch_logits"] - p["branch_logits"].max())
    w = w / w.sum()
    out["mlp2_lhsT"] = _bf(_pack_cols(p["mlp_w2"].T * w[1], 8))
    out["mlp2_b"] = _f32(_pack_cols(p["mlp_b2"].reshape(D, 1) * w[1], DD))
    out["w0_col"] = _f32(np.full((128, 1), w[0], np.float32))

    out["ident_bf"] = _bf(np.eye(128, dtype=np.float32))
    out["ident_f32"] = _f32(np.eye(128, dtype=np.float32))
    out["ones_f32r"] = _f32(np.ones((128, 1), np.float32))
    out["ones_bf"] = _bf(np.ones((128, 1), np.float32))
    return out


def _build():
    import contextlib

    import concourse.bacc as bacc
    import concourse.bass as bass
    import concourse.tile as tile
    from concourse import mybir

    F = mybir.dt.float32
    FR = mybir.dt.float32r
    BF = mybir.dt.bfloat16
    MUL = mybir.AluOpType.mult
    ADD = mybir.AluOpType.add
    SUB = mybir.AluOpType.subtract
    AF = mybir.ActivationFunctionType

    nc = bacc.Bacc("TRN2", target_bir_lowering=False, debug=False, num_devices=B)

    def param(name, shape, dtype=F):
        return nc.declare_dram_parameter(name, list(shape), dtype, isOutput=False)

    x_in = param("x", [L, D])
    rms_w_row = param("rms_w_row", [1, D])
    lconv_diag = param("lconv_diag", [128, 3 * DD, 128], BF)
    lconv_b = param("lconv_b", [128, DD, 1])
    lnc_w = param("lnc_w", [128, DD, 1])
    lnc_b = param("lnc_b", [128, DD, 1])
    inproj_lhsT = param("inproj_lhsT", [128, DD, 2 * DI], BF)
    mconv_diag = param("mconv_diag", [128, 4 * DT, 128], BF)
    mconv_b = param("mconv_b", [128, DT, 1])
    xproj_lhsT = param("xproj_lhsT", [128, DT, 96], FR)
    dtproj_lhsT = param("dtproj_lhsT", [R, DI], FR)
    dtproj_b = param("dtproj_b", [128, DT, 1])
    A_cols = param("A_cols", [128, DT, N])
    ssm_D = param("ssm_D", [128, DT, 1])
    outproj_lhsT = param("outproj_lhsT", [128, DT, D], BF)
    lnp_w = param("lnp_w", [128, 2 * DD, 1])
    lnp_b = param("lnp_b", [128, 2 * DD, 1])
    pconv_sel = param("pconv_sel", [128, DD * 2 * 3, 128], BF)
    pconv_b = param("pconv_b", [128, DD, 1])
    mlp1_lhsT = param("mlp1_lhsT", [128, DD, 4 * D], BF)
    mlp1_b = param("mlp1_b", [128, 8, 1])
    mlp2_lhsT = param("mlp2_lhsT", [128, 8, D], BF)
    mlp2_b = param("mlp2_b", [128, DD, 1])
    w0_col = param("w0_col", [128, 1])
    ident_bf = param("ident_bf", [128, 128], BF)
    ident_f32 = param("ident_f32", [128, 128])
    ones_f32r = param("ones_f32r", [128, 1], FR)
    ones_bf = param("ones_bf", [128, 1], BF)

    out_dram = nc.declare_dram_parameter("out", [L, D], F, isOutput=True)
    DBG = bool(int(os.environ.get("KDBG", "0")))
    if DBG:
        dbg_sz = nc.declare_dram_parameter("dbg_sz", [128, DT, L], BF, isOutput=True)
        dbg_gf = nc.declare_dram_parameter("dbg_gf", [128, DT, L], BF, isOutput=True)
        dbg_gb = nc.declare_dram_parameter("dbg_gb", [128, DT, L], BF, isOutput=True)
        dbg_xs = nc.declare_dram_parameter("dbg_xs", [128, 2 * DD, L], BF, isOutput=True)

    bc_dram = {d: nc.dram_tensor(f"bc_{d}", [64, L], BF) for d in ("f", "b")}
    db_dram = nc.dram_tensor("db_scr", [4, 128, DT * L], BF)
    xmd_dram = {d: nc.dram_tensor(f"xmd_{d}", [128, DT * L], BF) for d in ("f", "b")}
    stat_dram = nc.dram_tensor("stat_scr", [8, L], F)
    statbf_dram = nc.dram_tensor("statbf_scr", [4, L], BF)
    mlpT_dram = nc.dram_tensor("mlpT_scr", [L, D], BF)

    def brow(dram_t, off, ncols):
        ap = dram_t.ap()
        return bass.AP(tensor=ap.tensor, offset=off, ap=[[0, 128], [1, ncols]])

    def strided(ap2d, off, stride, count):
        # [128, count] view of a flat [128, X] tile AP at column offset/stride
        return bass.AP(tensor=ap2d.tensor, offset=ap2d.offset + off,
                       ap=[list(ap2d.ap[0]), [stride, count]])

    def bcast2(ap2d, ncols):
        # [128, 2, ncols] view of a [128, ncols] tile, broadcast on mid dim
        return bass.AP(tensor=ap2d.tensor, offset=ap2d.offset,
                       ap=[list(ap2d.ap[0]), [0, 2], [1, ncols]])

    def win3(ap2d, th_unused=None):
        # [128, 2, 1024] data view of a flat [128, 2050] window tile
        return ap2d[:, 0:2050].rearrange("p (a l) -> p a l", a=2)[:, :, 0:HL]

    with tile.TileContext(nc) as tc, contextlib.ExitStack() as ctx:
        consts = ctx.enter_context(tc.tile_pool(name="consts", bufs=1))
        wtmp = ctx.enter_context(tc.tile_pool(name="wtmp", bufs=1))
        big = ctx.enter_context(tc.tile_pool(name="big", bufs=1))
        sm = ctx.enter_context(tc.tile_pool(name="sm", bufs=4))
        pa = ctx.enter_context(tc.tile_pool(name="pa", bufs=2))
        pbz = ctx.enter_context(tc.tile_pool(name="pbz", bufs=4))
        pbc = ctx.enter_context(tc.tile_pool(name="pbc", bufs=3))
        pw8 = ctx.enter_context(tc.tile_pool(name="pw8", bufs=1))
        ps = ctx.enter_context(tc.tile_pool(name="ps", bufs=2, space="PSUM"))
        psy = ctx.enter_context(tc.tile_pool(name="psy", bufs=1, space="PSUM"))

        def load_const(name, prm, shape, dtype, pool=consts):
            t = pool.tile(shape, dtype, tag=name, name=name)
            nc.sync.dma_start(out=t[:], in_=prm.ap())
            return t

        c_ident_bf = load_const("c_ident_bf", ident_bf, [128, 128], BF)
        c_ones_fr = load_const("c_ones_fr", ones_f32r, [128, 1], FR)
        c_ones_bf = load_const("c_ones_bf", ones_bf, [128, 1], BF)
        c_xproj = load_const("c_xproj", xproj_lhsT, [128, DT, 96], FR)
        c_dtproj = load_const("c_dtproj", dtproj_lhsT, [R, DI], FR)
        c_outproj = load_const("c_outproj", outproj_lhsT, [128, DT, D], BF)
        c_A = load_const("c_A", A_cols, [128, DT, N], F)
        c_lconv_b = load_const("c_lconv_b", lconv_b, [128, DD, 1], F)
        c_lnc_w = load_const("c_lnc_w", lnc_w, [128, DD, 1], F)
        c_lnc_b = load_const("c_lnc_b", lnc_b, [128, DD, 1], F)
        c_mconv_b = load_const("c_mconv_b", mconv_b, [128, DT, 1], F)
        c_dtproj_b = load_const("c_dtproj_b", dtproj_b, [128, DT, 1], F)
        c_ssmD = load_const("c_ssmD", ssm_D, [128, DT, 1], F)
        c_lnp_w = load_const("c_lnp_w", lnp_w, [128, 2 * DD, 1], F)
        c_lnp_b = load_const("c_lnp_b", lnp_b, [128, 2 * DD, 1], F)
        c_pconv_b = load_const("c_pconv_b", pconv_b, [128, DD, 1], F)
        c_mlp1_b = load_const("c_mlp1_b", mlp1_b, [128, 8, 1], F)
        c_mlp2_b = load_const("c_mlp2_b", mlp2_b, [128, DD, 1], F)
        c_w0 = load_const("c_w0", w0_col, [128, 1], F)
        c_rmsw = consts.tile([128, D], F, tag="c_rmsw", name="c_rmsw")
        nc.sync.dma_start(out=c_rmsw[:], in_=brow(rms_w_row, 0, D))
        eps6 = consts.tile([128, 1], F, tag="eps6", name="eps6")
        nc.vector.memset(eps6[:], 1e-6)
        eps5 = consts.tile([128, 1], F, tag="eps5", name="eps5")
        nc.vector.memset(eps5[:], 1e-5)

        def wload(prm, shape, dtype, nm, tag="wbig"):
            t = wtmp.tile(shape, dtype, tag=tag, name=nm)
            nc.sync.dma_start(out=t[:], in_=prm.ap())
            return t

        xnT_pad = big.tile([128, DD, L + 2], BF, tag="xnT8", name="xnT_pad")
        sz = big.tile([128, DT, L], BF, tag="sz16", name="sz")
        hedge = big.tile([128, 2 * N, 2], F, tag="hedge", name="hedge")

        # manual ring buffers for the scan phase
        NAW = 2
        NBW = 2
        NHW = 2
        aw_ring = [big.tile([128, 2050], BF, tag=f"aw{j}", name=f"aw{j}")
                   for j in range(NAW)]
        bw_ring = [big.tile([128, 2050], BF, tag=f"bw{j}", name=f"bw{j}")
                   for j in range(NBW)]
        hw_ring = [big.tile([128, 2050], BF, tag=f"hw{j}", name=f"hw{j}")
                   for j in range(NHW)]
        for j in range(NAW):
            nc.vector.memset(aw_ring[j][:, HL:HL + 1], 0.0)
            nc.vector.memset(aw_ring[j][:, 2049:2050], 0.0)

        # ---- A1+A2: rmsnorm fused with transpose ---------------------------
        nc.vector.memset(xnT_pad[:, :, 0:1], 0.0)
        nc.vector.memset(xnT_pad[:, :, L + 1:L + 2], 0.0)
        for tt in range(NT):
            x_t = pa.tile([128, 512], F, tag="a_t", name="x_t")
            nc.gpsimd.dma_start(out=x_t[:, :D], in_=x_in.ap()[tt * 128:(tt + 1) * 128, :])
            sq = pbz.tile([128, 512], BF, tag="sq2", name="sq", bufs=1)
            ssq = sm.tile([128, 1], F, tag="ssq", name="ssq")
            nc.scalar.activation(sq[:, :D], x_t[:, :D], AF.Square, accum_out=ssq[:])
            rsq = sm.tile([128, 1], F, tag="rsq", name="rsq")
            nc.scalar.activation(rsq[:], ssq[:], AF.Sqrt, bias=eps6[:], scale=1.0 / D)
            nc.vector.reciprocal(rsq[:], rsq[:])
            xn_t = pbz.tile([128, 512], BF, tag="xn_t", name="xn_t", bufs=1)
            nc.scalar.activation(xn_t[:, :D], x_t[:, :D], AF.Copy, scale=rsq[:])
            nc.vector.tensor_tensor(out=xn_t[:, :D], in0=xn_t[:, :D], in1=c_rmsw[:], op=MUL)
            for dd in range(DD):
                pt = ps.tile([128, 512], BF, tag="convB", name="pt", bufs=1)
                nc.tensor.transpose(pt[:, :128], xn_t[:, dd * 128:(dd + 1) * 128], c_ident_bf[:])
                nc.vector.tensor_copy(out=xnT_pad[:, dd, 1 + tt * 128:1 + (tt + 1) * 128],
                                      in_=pt[:, :128])

        # ---- A3: local conv + square --------------------------------------
        c_lconv = wload(lconv_diag, [128, 3 * DD, 128], BF, "c_lconv")
        xc = big.tile([128, DD, L], BF, tag="beta16x", name="xc")
        xc2 = big.tile([128, DD, L], BF, tag="xc2mix", name="xc2")
        for dd in range(DD):
            for c in range(CH):
                pc = ps.tile([128, 512], F, tag="convA", name="pc")
                for k in range(3):
                    nc.tensor.matmul(pc[:], c_lconv[:, k * DD + dd, :],
                                     xnT_pad[:, dd, k + c * 512:k + c * 512 + 512],
                                     start=(k == 0), stop=(k == 2))
                nc.scalar.activation(xc[:, dd, c * 512:(c + 1) * 512], pc[:], AF.Identity,
                                     bias=c_lconv_b[:, dd, :])
                nc.scalar.activation(xc2[:, dd, c * 512:(c + 1) * 512], pc[:], AF.Square,
                                     bias=c_lconv_b[:, dd, :])

        # ---- A4: LNc stats + apply ----------------------------------------
        for si, src in ((0, xc), (1, xc2)):
            for c in range(CH):
                pst = ps.tile([1, 512], F, tag="stat", name="pst", bufs=1)
                for dd in range(DD):
                    nc.tensor.matmul(pst[:], c_ones_bf[:],
                                     src[:, dd, c * 512:(c + 1) * 512],
                                     start=(dd == 0), stop=(dd == DD - 1))
                sev = sm.tile([1, 512], F, tag="sev", name="sev")
                nc.vector.tensor_copy(out=sev[:], in_=pst[:])
                nc.sync.dma_start(out=stat_dram.ap()[si, c * 512:(c + 1) * 512].unsqueeze(0),
                                  in_=sev[:])
        st = sm.tile([128, 2, 16], F, tag="st", name="st")
        nc.sync.dma_start(out=st[:], in_=stat_dram.ap()[0:2, :].rearrange("s (p i) -> p s i", p=128))
        mu = sm.tile([128, 16], F, tag="mu", name="mu")
        nc.vector.tensor_scalar_mul(mu[:], st[:, 0, :], 1.0 / D)
        var = sm.tile([128, 16], F, tag="var", name="var")
        nc.vector.tensor_tensor(out=var[:], in0=mu[:], in1=mu[:], op=MUL)
        qv = sm.tile([128, 16], F, tag="qv", name="qv")
        nc.vector.tensor_scalar_mul(qv[:], st[:, 1, :], 1.0 / D)
        nc.vector.tensor_tensor(out=var[:], in0=qv[:], in1=var[:], op=SUB)
        rstd = sm.tile([128, 16], BF, tag="rstd", name="rstd")
        with nc.allow_low_precision(reason="rstd/nu are consumed as bf16 broadcasts"):
            nc.scalar.activation(rstd[:], var[:], AF.Sqrt, bias=eps5[:])
            nc.vector.reciprocal(rstd[:], rstd[:])
            nu = sm.tile([128, 16], BF, tag="nu", name="nu")
            nc.vector.tensor_tensor(out=nu[:], in0=mu[:], in1=rstd[:], op=MUL)
        nc.sync.dma_start(out=statbf_dram.ap()[0, :].rearrange("(p i) -> p i", p=128), in_=rstd[:])
        nc.sync.dma_start(out=statbf_dram.ap()[1, :].rearrange("(p i) -> p i", p=128), in_=nu[:])
        xcsT = big.tile([128, DD, L], BF, tag="delta16x", name="xcsT")
        for hh in range(2):
            rs_bc = pw8.tile([128, HL], BF, tag="t8w", name="rs_bc")
            nc.sync.dma_start(out=rs_bc[:], in_=brow(statbf_dram, hh * HL, HL))
            nu_bc = pw8.tile([128, HL], BF, tag="t8x", name="nu_bc")
            nc.sync.dma_start(out=nu_bc[:], in_=brow(statbf_dram, L + hh * HL, HL))
            for dd in range(DD):
                t1 = pw8.tile([128, HL], BF, tag="t8y", name="t1")
                hsl = slice(hh * HL, (hh + 1) * HL)
                nc.vector.tensor_tensor(out=t1[:], in0=xc[:, dd, hsl], in1=rs_bc[:], op=MUL)
                nc.vector.tensor_tensor(out=t1[:], in0=t1[:], in1=nu_bc[:], op=SUB)
                nc.scalar.activation(xcsT[:, dd, hsl], t1[:], AF.Silu, bias=c_lnc_b[:, dd, :],
                                     scale=c_lnc_w[:, dd, :])

        # ---- A5: in_proj ----------------------------------------------------
        c_inproj = wload(inproj_lhsT, [128, DD, 2 * DI], BF, "c_inproj")
        xmT_pad = big.tile([128, DT, L + 6], BF, tag="g16", name="xmT_pad")
        nc.vector.memset(xmT_pad[:, :, 0:3], 0.0)
        nc.vector.memset(xmT_pad[:, :, L + 3:L + 6], 0.0)
        for m in range(8):
            for c in range(CH):
                pc = ps.tile([128, 512], F, tag="convA", name="pc2")
                for kk in range(DD):
                    nc.tensor.matmul(pc[:], c_inproj[:, kk, m * 128:(m + 1) * 128],
                                     xcsT[:, kk, c * 512:(c + 1) * 512],
                                     start=(kk == 0), stop=(kk == DD - 1))
                if m < 4:
                    nc.scalar.copy(xmT_pad[:, m, 3 + c * 512:3 + (c + 1) * 512], pc[:])
                else:
                    nc.scalar.activation(sz[:, m - 4, c * 512:(c + 1) * 512], pc[:], AF.Silu)

        # ---- A6(d): mamba conv + x_proj + dt/beta/xmd ----------------------
        c_mconv = wload(mconv_diag, [128, 4 * DT, 128], BF, "c_mconv2", tag="wbig2")

        def stage_a6(d):
            xm_bf = big.tile([128, DT, L], BF, tag="xs16", name=f"xm_bf_{d}")
            dtT = big.tile([R, L], FR, tag="dtT", name=f"dtT_{d}")
            for c in range(CH):
                xm_fr = pa.tile([128, DT, 512], FR, tag="xm_fr", name="xm_fr", bufs=1)
                for dt in range(DT):
                    pc = ps.tile([128, 512], F, tag="convA", name="pc3")
                    for k in range(4):
                        off = (k if d == "f" else 6 - k) + c * 512
                        nc.tensor.matmul(pc[:], c_mconv[:, k * DT + dt, :],
                                         xmT_pad[:, dt, off:off + 512],
                                         start=(k == 0), stop=(k == 3))
                    nc.scalar.activation(xm_fr[:, dt, :], pc[:], AF.Silu, bias=c_mconv_b[:, dt, :])
                    nc.scalar.activation(xm_bf[:, dt, c * 512:(c + 1) * 512], pc[:], AF.Silu,
                                         bias=c_mconv_b[:, dt, :])
                psx = ps.tile([96, 512], F, tag="stat", name="psx", bufs=1)
                for dt in range(DT):
                    nc.tensor.matmul(psx[:], c_xproj[:, dt, :], xm_fr[:, dt, :],
                                     start=(dt == 0), stop=(dt == DT - 1))
                nc.scalar.copy(dtT[:, c * 512:(c + 1) * 512], psx[0:R, :])
                bc_ev = pbz.tile([64, 512], BF, tag="bc_ev", name="bc_ev", bufs=1)
                nc.scalar.copy(bc_ev[0:32, :], psx[32:64, :])
                nc.scalar.copy(bc_ev[32:64, :], psx[64:96, :])
                nc.sync.dma_start(out=bc_dram[d].ap()[:, c * 512:(c + 1) * 512], in_=bc_ev[:])
            drow = 0 if d == "f" else 2
            for ms, cs in (((0, 1), (0, 1)), ((0, 1), (2, 3)), ((2, 3), (0, 1, 2, 3))):
                e1t = big.tile([128, 2, L], BF, tag="xc2mix", name=f"e1t_{d}")
                for mi, m in enumerate(ms):
                    for c in cs:
                        pc = ps.tile([128, 512], F, tag="convA", name="pc4")
                        nc.tensor.matmul(pc[:], c_dtproj[:, m * 128:(m + 1) * 128],
                                         dtT[:, c * 512:(c + 1) * 512], start=True, stop=True)
                        nc.scalar.activation(e1t[:, mi, c * 512:(c + 1) * 512], pc[:], AF.Exp,
                                             bias=c_dtproj_b[:, m, :])
                for mi, m in enumerate(ms):
                    for c in cs:
                        dev = pbz.tile([128, 512], BF, tag="dev", name="dev", bufs=2)
                        nc.scalar.activation(dev[:], e1t[:, mi, c * 512:(c + 1) * 512], AF.Ln,
                                             bias=1.0)
                        nc.sync.dma_start(
                            out=db_dram.ap()[drow, :, m * L + c * 512:m * L + (c + 1) * 512],
                            in_=dev[:])
                        bev = pbz.tile([128, 512], BF, tag="bev", name="bev", bufs=2)
                        nc.vector.tensor_tensor(out=bev[:], in0=dev[:],
                                                in1=xm_bf[:, m, c * 512:(c + 1) * 512], op=MUL)
                        nc.sync.dma_start(
                            out=db_dram.ap()[drow + 1, :, m * L + c * 512:m * L + (c + 1) * 512],
                            in_=bev[:])
            for dt in range(DT):
                for c in range(CH):
                    xev = pbz.tile([128, 512], BF, tag="xev", name="xev", bufs=2)
                    nc.scalar.activation(xev[:], xm_bf[:, dt, c * 512:(c + 1) * 512],
                                         AF.Copy, scale=c_ssmD[:, dt, :])
                    nc.sync.dma_start(
                        out=xmd_dram[d].ap()[:, dt * L + c * 512:dt * L + (c + 1) * 512],
                        in_=xev[:])

        # ---- scan phase: one (direction, pair) window group ----------------
        gtiles = {}
        ring_idx = [0, 0, 0]  # aw, bw, hw

        def stage_scan_pair(d, pair):
            drow = 0 if d == "f" else 2
            if pair == 0:
                tag = "gdir" if d == "f" else "g16"
                gtiles[d] = big.tile([128, DT, L], BF, tag=tag, name=f"g_{d}")
            g = gtiles[d]
            delta_p = big.tile([128, 2, L], BF, tag="delta16x", name=f"delta_{d}{pair}")
            beta_p = big.tile([128, 2, L], BF, tag="beta16x", name=f"beta_{d}{pair}")
            ths = (0, 1) if d == "f" else (1, 0)
            for sh in (ths[0], ths[1]):
                hs = slice(sh * HL, (sh + 1) * HL)
                nc.sync.dma_start(out=delta_p[:, :, hs], in_=db_dram.ap()[
                    drow, :, 2 * pair * L:(2 * pair + 2) * L].rearrange(
                        "p (a l) -> p a l", a=2)[:, :, hs])
                nc.sync.dma_start(out=beta_p[:, :, hs], in_=db_dram.ap()[
                    drow + 1, :, 2 * pair * L:(2 * pair + 2) * L].rearrange(
                        "p (a l) -> p a l", a=2)[:, :, hs])
            for thi, th in enumerate(ths):
                # prezero bw link columns at window start (first th only needs
                # 0-link; second th overwrites the link with the carried state)
                if thi == 0:
                    for j in range(NBW):
                        nc.vector.memset(bw_ring[j][:, HL:HL + 1], 0.0)
                        nc.vector.memset(bw_ring[j][:, 2049:2050], 0.0)
                yp = psy.tile([128, 2, HL], F, tag="ypair", name="yp")
                xmd_t = {}
                for i in range(2):
                    dt = pair * 2 + i
                    xmd_t[i] = pbz.tile([128, HL], BF, tag=f"xmd{i}", name="xmd_c", bufs=1)
                    nc.gpsimd.dma_start(
                        out=xmd_t[i][:],
                        in_=xmd_dram[d].ap()[:, dt * L + th * HL:dt * L + (th + 1) * HL])
                for n in range(N):
                    slot = pair * N + n
                    B_bc = pbc.tile([128, HL], BF, tag="B_bc", name="B_bc", bufs=2)
                    nc.gpsimd.dma_start(out=B_bc[:], in_=brow(bc_dram[d], n * L + th * HL, HL))
                    C_bc = pbc.tile([128, HL], BF, tag="C_bc", name="C_bc", bufs=2)
                    nc.gpsimd.dma_start(out=C_bc[:],
                                        in_=brow(bc_dram[d], (N + n) * L + th * HL, HL))
                    aw = aw_ring[ring_idx[0] % NAW]; ring_idx[0] += 1
                    bw = bw_ring[ring_idx[1] % NBW]; ring_idx[1] += 1
                    hw = hw_ring[ring_idx[2] % NHW]; ring_idx[2] += 1
                    r_aw = win3(aw[:])
                    for i in range(2):
                        nc.scalar.activation(r_aw[:, i, :], delta_p[:, i, th * HL:(th + 1) * HL],
                                             AF.Exp, scale=c_A[:, pair * 2 + i, n:n + 1])
                    nc.vector.tensor_tensor(out=win3(bw[:]),
                                            in0=beta_p[:, :, th * HL:(th + 1) * HL],
                                            in1=bcast2(B_bc[:], HL), op=MUL)
                    if thi == 1:
                        src = hedge[:, slot, 1:2] if d == "f" else hedge[:, slot, 0:1]
                        nc.vector.tensor_copy(out=bw[:, HL:HL + 1], in_=src)
                        init = hedge[:, slot, 0:1] if d == "f" else hedge[:, slot, 1:2]
                    else:
                        init = 0.0
                    if d == "f":
                        nc.vector.tensor_tensor_scan(hw[:, 0:W2], aw[:, 0:W2], bw[:, 0:W2],
                                                     init, MUL, ADD)
                    else:
                        nc.vector.tensor_tensor_scan(hw[:, 0:W2][:, ::-1], aw[:, 0:W2][:, ::-1],
                                                     bw[:, 0:W2][:, ::-1], init, MUL, ADD)
                    if thi == 0:
                        off = 1023 if d == "f" else 0
                        nc.vector.tensor_copy(out=hedge[:, slot, 0:2],
                                              in_=strided(hw[:], off, 1025, 2))
                    nc.vector.tensor_tensor(out=win3(hw[:]), in0=win3(hw[:]),
                                            in1=bcast2(C_bc[:], HL), op=MUL)
                    for i in range(2):
                        for c2 in range(2):
                            base = i * 1025 + c2 * 512
                            nc.tensor.matmul(yp[:, i, c2 * 512:(c2 + 1) * 512], c_ident_bf[:],
                                             hw[:, base:base + 512],
                                             start=(n == 0), stop=False)
                for i in range(2):
                    for c2 in range(2):
                        nc.tensor.matmul(yp[:, i, c2 * 512:(c2 + 1) * 512], c_ident_bf[:],
                                         xmd_t[i][:, c2 * 512:(c2 + 1) * 512],
                                         start=False, stop=True)
                nc.vector.tensor_tensor(
                    out=g[:, 2 * pair:2 * pair + 2, th * HL:(th + 1) * HL],
                    in0=yp[:, :, :],
                    in1=sz[:, 2 * pair:2 * pair + 2, th * HL:(th + 1) * HL], op=MUL)

        # ---- C/E: out_proj + residual --------------------------------------
        def stage_outproj(d, xs):
            g = gtiles[d]
            evict_dve = (d == "b")
            base = 0 if d == "f" else DD
            for m in range(DD):
                for c in range(CH):
                    pc = ps.tile([128, 512], F, tag="convA", name="pc5")
                    for dt in range(DT):
                        nc.tensor.matmul(pc[:], c_outproj[:, dt, m * 128:(m + 1) * 128],
                                         g[:, dt, c * 512:(c + 1) * 512],
                                         start=(dt == 0), stop=False)
                    nc.tensor.matmul(pc[:], c_ident_bf[:],
                                     xnT_pad[:, m, 1 + c * 512:1 + (c + 1) * 512],
                                     start=False, stop=True)
                    if evict_dve:
                        nc.vector.tensor_copy(out=xs[:, base + m, c * 512:(c + 1) * 512],
                                              in_=pc[:])
                    else:
                        nc.scalar.copy(xs[:, base + m, c * 512:(c + 1) * 512], pc[:])

        # ---- F3: MLP -> mlpT rows in DRAM ----------------------------------
        def stage_mlp():
            c_mlp1 = wload(mlp1_lhsT, [128, DD, 4 * D], BF, "c_mlp1")
            c_mlp2 = wload(mlp2_lhsT, [128, 8, D], BF, "c_mlp2", tag="wbig2")
            for c in range(CH):
                h1 = big.tile([128, 8, 512], BF, tag="xc2mix", name="h1")
                for m in range(8):
                    pc = ps.tile([128, 512], F, tag="convA", name="pc7")
                    for kk in range(DD):
                        nc.tensor.matmul(pc[:], c_mlp1[:, kk, m * 128:(m + 1) * 128],
                                         xnT_pad[:, kk, 1 + c * 512:1 + (c + 1) * 512],
                                         start=(kk == 0), stop=(kk == DD - 1))
                    nc.scalar.activation(h1[:, m, :], pc[:], AF.Gelu, bias=c_mlp1_b[:, m, :])
                mlpc = pbz.tile([128, DD, 512], BF, tag="b_t", name="mlpc", bufs=2)
                for m2 in range(DD):
                    pc = ps.tile([128, 512], F, tag="convA", name="pc8")
                    for mk in range(8):
                        nc.tensor.matmul(pc[:], c_mlp2[:, mk, m2 * 128:(m2 + 1) * 128],
                                         h1[:, mk, :], start=(mk == 0), stop=(mk == 7))
                    nc.scalar.activation(mlpc[:, m2, :], pc[:], AF.Identity,
                                         bias=c_mlp2_b[:, m2, :])
                for q in range(4):
                    tt = c * 4 + q
                    mt = pbz.tile([128, 512], BF, tag="z_t", name="mt", bufs=2)
                    for m2 in range(DD):
                        pt3 = ps.tile([128, 512], BF, tag="convB", name="pt3", bufs=1)
                        nc.tensor.transpose(pt3[:, :128], mlpc[:, m2, q * 128:(q + 1) * 128],
                                            c_ident_bf[:])
                        nc.scalar.copy(mt[:, m2 * 128:(m2 + 1) * 128], pt3[:, :128])
                    nc.sync.dma_start(out=mlpT_dram.ap()[tt * 128:(tt + 1) * 128, :],
                                      in_=mt[:, :D])

        # ================= emission schedule =================
        stage_a6("f")
        stage_scan_pair("f", 0)
        stage_a6("b")
        xs = big.tile([128, 2 * DD, L], BF, tag="xs16", name="xs")
        stage_scan_pair("f", 1)
        stage_outproj("f", xs)
        xs2 = big.tile([128, 2 * DD, L], BF, tag="gdir", name="xs2")
        for dt in range(DD):
            nc.scalar.activation(xs2[:, dt, :], xs[:, dt, :], AF.Square)
        stage_scan_pair("b", 0)
        stage_mlp()
        stage_scan_pair("b", 1)
        stage_outproj("b", xs)
        if DBG:
            nc.sync.dma_start(out=dbg_sz.ap(), in_=sz[:])
            nc.sync.dma_start(out=dbg_gf.ap(), in_=gtiles["f"][:])
            nc.sync.dma_start(out=dbg_gb.ap(), in_=gtiles["b"][:])
            nc.sync.dma_start(out=dbg_xs.ap(), in_=xs[:])

        # ---- F1: LNp ---------------------------------------------------------
        for dt in range(DD, 2 * DD):
            nc.scalar.activation(xs2[:, dt, :], xs[:, dt, :], AF.Square)
        for si, src in ((4, xs), (5, xs2)):
            for c in range(CH):
                pst = ps.tile([1, 512], F, tag="stat", name="pst2", bufs=1)
                for dt in range(2 * DD):
                    nc.tensor.matmul(pst[:], c_ones_bf[:], src[:, dt, c * 512:(c + 1) * 512],
                                     start=(dt == 0), stop=(dt == 2 * DD - 1))
                sev2 = sm.tile([1, 512], F, tag="sev", name="sev2")
                nc.vector.tensor_copy(out=sev2[:], in_=pst[:])
                nc.sync.dma_start(out=stat_dram.ap()[si, c * 512:(c + 1) * 512].unsqueeze(0),
                                  in_=sev2[:])
        stp = sm.tile([128, 2, 16], F, tag="st", name="stp")
        nc.sync.dma_start(out=stp[:], in_=stat_dram.ap()[4:6, :].rearrange("s (p i) -> p s i", p=128))
        mup = sm.tile([128, 16], F, tag="mu", name="mup")
        nc.vector.tensor_scalar_mul(mup[:], stp[:, 0, :], 1.0 / (2 * D))
        varp = sm.tile([128, 16], F, tag="var", name="varp")
        nc.vector.tensor_tensor(out=varp[:], in0=mup[:], in1=mup[:], op=MUL)
        qp = sm.tile([128, 16], F, tag="qv", name="qp")
        nc.vector.tensor_scalar_mul(qp[:], stp[:, 1, :], 1.0 / (2 * D))
        nc.vector.tensor_tensor(out=varp[:], in0=qp[:], in1=varp[:], op=SUB)
        rstdp = sm.tile([128, 16], BF, tag="rstd", name="rstdp")
        with nc.allow_low_precision(reason="rstd/nu are consumed as bf16 broadcasts"):
            nc.scalar.activation(rstdp[:], varp[:], AF.Sqrt, bias=eps5[:])
            nc.vector.reciprocal(rstdp[:], rstdp[:])
            nup = sm.tile([128, 16], BF, tag="nu", name="nup")
            nc.vector.tensor_tensor(out=nup[:], in0=mup[:], in1=rstdp[:], op=MUL)
        nc.sync.dma_start(out=statbf_dram.ap()[2, :].rearrange("(p i) -> p i", p=128), in_=rstdp[:])
        nc.sync.dma_start(out=statbf_dram.ap()[3, :].rearrange("(p i) -> p i", p=128), in_=nup[:])
        xsn_pad = big.tile([128, 2 * DD, L + 2], BF, tag="g16", name="xsn_pad")
        nc.vector.memset(xsn_pad[:, :, 0:1], 0.0)
        nc.vector.memset(xsn_pad[:, :, L + 1:L + 2], 0.0)
        for hh in range(2):
            rsp_bc = pw8.tile([128, HL], BF, tag="t8w", name="rsp_bc")
            nc.sync.dma_start(out=rsp_bc[:], in_=brow(statbf_dram, 2 * L + hh * HL, HL))
            nup_bc = pw8.tile([128, HL], BF, tag="t8x", name="nup_bc")
            nc.sync.dma_start(out=nup_bc[:], in_=brow(statbf_dram, 3 * L + hh * HL, HL))
            for dt in range(2 * DD):
                t1 = pw8.tile([128, HL], BF, tag="t8y", name="t1p")
                hsl = slice(hh * HL, (hh + 1) * HL)
                nc.vector.tensor_tensor(out=t1[:], in0=xs[:, dt, hsl], in1=rsp_bc[:], op=MUL)
                nc.vector.tensor_tensor(out=t1[:], in0=t1[:], in1=nup_bc[:], op=SUB)
                nc.scalar.activation(xsn_pad[:, dt, 1 + hh * HL:1 + (hh + 1) * HL], t1[:],
                                     AF.Identity, bias=c_lnp_b[:, dt, :], scale=c_lnp_w[:, dt, :])

        # ---- F2: post conv ---------------------------------------------------
        c_psel = wload(pconv_sel, [128, DD * 2 * 3, 128], BF, "c_psel")
        mixer = big.tile([128, DD, L], BF, tag="xc2mix", name="mixer")
        for gt in range(DD):
            for c in range(CH):
                pc = ps.tile([128, 512], F, tag="convA", name="pc6")
                first = True
                for jj in range(2):
                    ct = 2 * gt + jj
                    for k in range(3):
                        nc.tensor.matmul(pc[:], c_psel[:, (gt * 2 + jj) * 3 + k, :],
                                         xsn_pad[:, ct, k + c * 512:k + c * 512 + 512],
                                         start=first, stop=(jj == 1 and k == 2))
                        first = False
                nc.scalar.activation(mixer[:, gt, c * 512:(c + 1) * 512], pc[:], AF.Silu,
                                     bias=c_pconv_b[:, gt, :])

        # ---- F4: transpose back + combine + store ----------------------------
        for tt in range(NT):
            mixT = pbz.tile([128, 256], BF, tag="mxT", name="mixT", bufs=3)
            mlpT = pbz.tile([128, 256], BF, tag="mpT", name="mlpT", bufs=3)
            nc.sync.dma_start(out=mlpT[:], in_=mlpT_dram.ap()[tt * 128:(tt + 1) * 128, :])
            for dd in range(DD):
                pt = ps.tile([128, 512], BF, tag="convA", name="ptb")
                nc.tensor.transpose(pt[:, :128], mixer[:, dd, tt * 128:(tt + 1) * 128],
                                    c_ident_bf[:])
                nc.scalar.copy(mixT[:, dd * 128:(dd + 1) * 128], pt[:, :128])
            x_t2 = pa.tile([128, 512], F, tag="a_t", name="x_t2")
            nc.gpsimd.dma_start(out=x_t2[:, :D], in_=x_in.ap()[tt * 128:(tt + 1) * 128, :])
            o_t = pbz.tile([128, 256], F, tag="o_t", name="o_t", bufs=2)
            nc.vector.tensor_scalar_mul(o_t[:], mixT[:], c_w0[:])
            nc.vector.tensor_tensor(out=o_t[:], in0=o_t[:], in1=x_t2[:, :D], op=ADD)
            nc.vector.tensor_tensor(out=o_t[:], in0=o_t[:], in1=mlpT[:], op=ADD)
            nc.sync.dma_start(out=out_dram.ap()[tt * 128:(tt + 1) * 128, :], in_=o_t[:])

    nc.compile()
    return nc


def kernel(**inputs):
    global _BUILT
    from concourse.bass_utils import run_bass_kernel_spmd

    if _BUILT is None:
        _BUILT = _build()
    nc = _BUILT

    w = _host_prep(inputs)
    x = np.asarray(inputs["x"], dtype=np.float32)
    in_maps = []
    for b in range(B):
        m = dict(w)
        m["x"] = np.ascontiguousarray(x[b])
        in_maps.append(m)
    res = run_bass_kernel_spmd(nc, in_maps, list(range(B)))
    out = np.stack([res.results[b]["out"] for b in range(B)], axis=0)
    return out.astype(np.float32)


if __name__ == "__main__":
    rng = np.random.default_rng(0)
    fake = {
        "x": rng.standard_normal((B, L, D), dtype=np.float32),
        "rms_w": np.ones(D, np.float32),
        "local_conv_w": rng.standard_normal((D, 1, 3), dtype=np.float32) * 0.3,
        "local_conv_b": np.zeros(D, np.float32),
        "lnc_w": np.ones(D, np.float32),
        "lnc_b": np.zeros(D, np.float32),
        "in_proj_w": rng.standard_normal((2 * DI, D), dtype=np.float32) * 0.02,
        "conv1d_w": rng.standard_normal((DI, 1, 4), dtype=np.float32) * 0.3,
        "conv1d_b": np.zeros(DI, np.float32),
        "x_proj_w": rng.standard_normal((R + 2 * N, DI), dtype=np.float32) * 0.02,
        "dt_proj_w": rng.standard_normal((DI, R), dtype=np.float32) * 0.1,
        "dt_proj_b": np.full(DI, -4.0, np.float32),
        "A_log": np.log(np.tile(np.arange(1, N + 1, dtype=np.float32), (DI, 1))),
        "ssm_D": np.ones(DI, np.float32),
        "out_proj_w": rng.standard_normal((D, DI), dtype=np.float32) * 0.02,
        "lnp_w": np.ones(2 * D, np.float32),
        "lnp_b": np.zeros(2 * D, np.float32),
        "post_conv_w": rng.standard_normal((D, 2, 3), dtype=np.float32) * 0.3,
        "post_conv_b": np.zeros(D, np.float32),
        "mlp_w1": rng.standard_normal((4 * D, D), dtype=np.float32) * 0.02,
        "mlp_b1": np.zeros(4 * D, np.float32),
        "mlp_w2": rng.standard_normal((D, 4 * D), dtype=np.float32) * 0.02,
        "mlp_b2": np.zeros(D, np.float32),
        "branch_logits": np.array([1.0, 0.1], np.float32),
    }
    out = kernel(**fake)
    print("kernel ran, out shape", out.shape, "finite:", bool(np.isfinite(out).all()))
